# revision 18
# baseline (speedup 1.0000x reference)
"""Trainium2 Bass kernel for a dense decoder block (LN->MHA->res, LN->FFN->res).

Sharding (8 cores, one NEFF, SPMD-uniform addressing):
  - LN1 token-parallel (512-token chunk/core) -> AllGather of normalized acts
    quantized to fp8e4m3 (x32-scaled weights keep fp8 out of subnormals).
  - QKV + attention head-parallel (2 heads/core, causal, unstable softmax --
    exact because masked logits multiply to 0 post-exp).
  - AllToAll redistributes attention values (fp8): head-shards -> token-shards.
  - proj + residual + LN2 + FFN token-parallel with fp8 weights streamed.
  - LN affine params are folded into the following matmul weights on host.

All heavy GEMMs run in fp8e4m3 with MatmulPerfMode.DoubleRow (256-row
contraction per instruction, 0.5 cycles/output column -> 4x f32r MACs/cycle).
Weights are pre-scaled by 32 on host; PSUM drains apply 1/32 via the
activation-engine scale. Attention (scores/softmax/AV) runs in bf16.
LN statistics use the ones-matmul trick in f32r (1 cycle/row).
Activations stay channel-major [C, tokens]; v is produced token-major
directly by swapping matmul operands, so no transposes are needed.
"""

import math

import numpy as np
import ml_dtypes

import concourse.bass as bass
import concourse.mybir as mybir
import concourse.tile as tile
from concourse import bacc
from concourse import bass_utils

F32 = mybir.dt.float32
F32R = mybir.dt.float32r
BF16 = mybir.dt.bfloat16
F8 = mybir.dt.float8e4
AF = mybir.ActivationFunctionType
OP = mybir.AluOpType
PM = mybir.MatmulPerfMode

N_CORES = 8
B = 2
C = 2048
H = 16
HD = 128
F = 8192
NT = B * 2048                       # total tokens (B*T with T=2048)
H_PER_CORE = H // N_CORES           # 2
NCT = C // 128                      # 16 channel tiles
NB = C // 256                       # 8 DoubleRow contraction blocks
NFT = F // 128                      # 64 ffn tiles
NFB = F // 256                      # 32 ffn DoubleRow blocks
EPS = 1e-5
SCALE = 1.0 / math.sqrt(HD)
WS = 32.0                           # host-side weight scale (drains apply 1/WS)
RWS = 1.0 / WS
GELU = AF.Gelu_apprx_tanh


def r32(ap):
    return ap.bitcast(F32R)


def _ln_finish(nc, pool_small, ps_sum, ps_ssq, n_tok, ncols, tagpfx, lp=None):
    """From broadcast sum/sumsq psums produce SBUF rstd/shift [128, ncols].

    With lp set, outputs are bf16 (feeding the bf16 apply path)."""
    odt = BF16 if lp is not None else F32
    mean = pool_small.tile([128, ncols], F32, tag=f"{tagpfx}_mean", name="mean")
    ex2 = pool_small.tile([128, ncols], F32, tag=f"{tagpfx}_ex2", name="ex2")
    nc.vector.tensor_scalar_mul(mean[:], ps_sum[:], 1.0 / n_tok)
    nc.vector.tensor_scalar_mul(ex2[:], ps_ssq[:], 1.0 / n_tok)
    msq = pool_small.tile([128, ncols], F32, tag=f"{tagpfx}_msq", name="msq")
    nc.vector.tensor_mul(msq[:], mean[:], mean[:])
    varp = pool_small.tile([128, ncols], F32, tag=f"{tagpfx}_varp", name="varp")
    nc.vector.scalar_tensor_tensor(varp[:], ex2[:], EPS, msq[:],
                                   op0=OP.add, op1=OP.subtract)
    std = pool_small.tile([128, ncols], F32, tag=f"{tagpfx}_std", name="std")
    nc.scalar.sqrt(std[:], varp[:])
    rstd_bc = pool_small.tile([128, ncols], odt, tag=f"{tagpfx}_rstd", name="rstd")
    if lp is not None:
        with lp.allow_low_precision(reason="rstd broadcast feeds fp8 path"):
            nc.vector.reciprocal(rstd_bc[:], std[:])
    else:
        nc.vector.reciprocal(rstd_bc[:], std[:])
    shift_bc = pool_small.tile([128, ncols], odt, tag=f"{tagpfx}_shift", name="shift")
    nc.vector.scalar_tensor_tensor(shift_bc[:], mean[:], -1.0, rstd_bc[:],
                                   op0=OP.mult, op1=OP.mult)
    return rstd_bc, shift_bc


def build_decoder(T=2048, collectives=True):
    """Build the SPMD decoder-block program for seq length T (2048 = real)."""
    NTOK = B * T
    CH = NTOK // N_CORES            # tokens per core chunk (512)
    NQS = max(1, T // 512)          # q slices of 512 per batch elem
    QS = min(512, T)
    NVT = NTOK // 128               # token-major v tiles (32)
    S_SUB = CH // 128               # 128-token subtiles per chunk (4)

    nc = bacc.Bacc("TRN2", target_bir_lowering=False, debug=False,
                   num_devices=N_CORES)

    # ---- I/O ----
    xt = nc.dram_tensor("xt", [C, CH], F32, kind="ExternalInput").ap()
    wq8 = nc.dram_tensor("wq8", [128, 2 * NB * 2, 128], F8, kind="ExternalInput").ap()
    wk8 = nc.dram_tensor("wk8", [128, 2 * NB * 2, 128], F8, kind="ExternalInput").ap()
    wv8 = nc.dram_tensor("wv8", [128, NB * 2, 256], F8, kind="ExternalInput").ap()
    bq = nc.dram_tensor("bq", [128, 2, 1], F32, kind="ExternalInput").ap()
    bk = nc.dram_tensor("bk", [128, 2, 1], F32, kind="ExternalInput").ap()
    bv_bc = nc.dram_tensor("bv_bc", [128, 256], F32, kind="ExternalInput").ap()
    wproj8 = nc.dram_tensor("wproj8", [128, NCT * NB * 2, 128], F8,
                            kind="ExternalInput").ap()
    bproj = nc.dram_tensor("bproj", [128, NCT, 1], F32, kind="ExternalInput").ap()
    wf18 = nc.dram_tensor("wf18", [NFT, 128, NB * 2, 128], F8,
                          kind="ExternalInput").ap()
    bf1 = nc.dram_tensor("bf1", [128, NFT, 1], F32, kind="ExternalInput").ap()
    wf28 = nc.dram_tensor("wf28", [NCT, 128, NFB * 2, 128], F8,
                          kind="ExternalInput").ap()
    bf2 = nc.dram_tensor("bf2", [128, NCT, 1], F32, kind="ExternalInput").ap()
    masks = nc.dram_tensor("masks", [128, 4, QS], BF16, kind="ExternalInput").ap()
    out = nc.dram_tensor("out", [C, CH], F32, kind="ExternalOutput").ap()

    RG = [list(range(N_CORES))]

    with tile.TileContext(nc) as tc:
        with tc.tile_pool(name="dram", bufs=1, space="DRAM") as dram, \
             tc.tile_pool(name="persist", bufs=1) as persist:
            n1_bounce = [dram.tile([C // 2, CH], F8, tag=f"n1_bounce{hh}",
                                   name="n1_bounce") for hh in range(2)]
            n1_full = [dram.tile([N_CORES * C // 2, CH], F8, tag=f"n1_full{hh}",
                                 name="n1_full", addr_space="Shared")
                       for hh in range(2)]
            a2a_in = [dram.tile([C // 2, CH], F8, tag=f"a2a_in{h}",
                                name="a2a_in") for h in range(2)]
            a2a_out = [dram.tile([C // 2, CH], F8, tag=f"a2a_out{h}",
                                 name="a2a_out") for h in range(2)]

            # x tiles are the critical path at t=0: issue their DMAs first.
            xt_view = xt.rearrange("(k p) t -> p k t", p=128)
            ones_sq = persist.tile([128, 128], F32, tag="ones_sq", name="ones_sq")
            ones_bf = persist.tile([128, 128], BF16, tag="ones_bf", name="ones_bf")
            nc.vector.memset(ones_sq[:], 1.0)
            nc.vector.tensor_copy(ones_bf[:], ones_sq[:])
            masks_sb = persist.tile([128, 4, QS], BF16, tag="masks", name="masks_sb")
            bq_sb = persist.tile([128, 2, 1], F32, tag="bq", name="bq_sb")
            bk_sb = persist.tile([128, 2, 1], F32, tag="bk", name="bk_sb")
            bv_sb = persist.tile([128, 256], F32, tag="bv", name="bv_sb")
            bproj_sb = persist.tile([128, NCT, 1], F32, tag="bproj", name="bproj_sb")
            bf1_sb = persist.tile([128, NFT, 1], F32, tag="bf1", name="bf1_sb")
            bf2_sb = persist.tile([128, NCT, 1], F32, tag="bf2", name="bf2_sb")

            # r1 survives proj -> final residual add; x survives LN1 -> proj.
            r1_sb = persist.tile([128, NCT, CH], F32, tag="r1", name="r1_sb")

            with tc.tile_pool(name="xpool", bufs=1) as xpool:
                # four separate tiles: a single x tile would serialize each
                # quarter's DMA behind the previous quarter's readers (the
                # dependency tracker is tile-granular)
                x_q = [xpool.tile([128, 4, CH], F32, tag=f"x_q{q}", name="x_q")
                       for q in range(4)]

                def x_tile(k):
                    return x_q[k // 4][:, k % 4, :]
                n2pool = tc.alloc_tile_pool(name="n2pool", bufs=1)
                n2_sb = n2pool.tile([128, NCT, CH], F8, tag="n2_sb",
                                    name="n2_sb")
                projw = tc.alloc_tile_pool(name="projw", bufs=1)
                wp_sb = projw.tile([128, NCT * NB * 2, 128], F8, tag="wp",
                                   name="wp_sb")
                wqkvp = tc.alloc_tile_pool(name="wqkv", bufs=1)
                wq_sb = wqkvp.tile([128, 2 * NB * 2, 128], F8, tag="wq",
                                   name="wq_sb")
                wk_sb = wqkvp.tile([128, 2 * NB * 2, 128], F8, tag="wk",
                                   name="wk_sb")
                wv_sb = wqkvp.tile([128, NB * 2, 256], F8, tag="wv",
                                   name="wv_sb")
                xbfpool = tc.alloc_tile_pool(name="xbfpool", bufs=1)
                x_bf = xbfpool.tile([128, NCT, CH], BF16, tag="x_bf", name="x_bf")

                # ================= Phase A: LN1 on own chunk =================
                with tc.tile_pool(name="lnA", bufs=3) as lnA, \
                     tc.tile_pool(name="lnA_small", bufs=1) as lnAs, \
                     tc.tile_pool(name="n1pool", bufs=2) as n1pool, \
                     tc.tile_pool(name="psA", bufs=1, space="PSUM") as psA:
                    ps_sum = psA.tile([128, CH], F32, tag="sum", name="ps_sum")
                    ps_ssq = psA.tile([128, CH], F32, tag="ssq", name="ps_ssq")
                    for q in range(4):
                        for hq in range(2):
                            nc.sync.dma_start(
                                x_q[q][:, 2 * hq:2 * hq + 2, :],
                                xt_view[:, 4 * q + 2 * hq:4 * q + 2 * hq + 2, :])
                    nc.sync.dma_start(wq_sb[:], wq8)
                    nc.sync.dma_start(wk_sb[:], wk8)
                    nc.sync.dma_start(wv_sb[:], wv8)
                    nc.sync.dma_start(bq_sb[:], bq)
                    nc.sync.dma_start(bk_sb[:], bk)
                    nc.sync.dma_start(bv_sb[:], bv_bc)
                    nc.sync.dma_start(masks_sb[:], masks)
                    nc.sync.dma_start(bproj_sb[:], bproj)
                    nc.sync.dma_start(bf1_sb[:], bf1)
                    nc.sync.dma_start(bf2_sb[:], bf2)
                    for k in range(NCT):
                        nc.scalar.activation(x_bf[:, k, :], x_tile(k),
                                             AF.Identity)
                        sq = lnA.tile([128, CH], BF16, tag="sq", name="sq")
                        nc.vector.tensor_mul(sq[:], x_bf[:, k, :], x_bf[:, k, :])
                        nc.tensor.matmul(ps_sum[:], ones_bf[:], x_bf[:, k, :],
                                         start=(k == 0), stop=(k == NCT - 1))
                        nc.tensor.matmul(ps_ssq[:], ones_bf[:], sq[:],
                                         start=(k == 0), stop=(k == NCT - 1))
                    rstd_bf, shift_bf = _ln_finish(nc, lnAs, ps_sum, ps_ssq,
                                                   C, CH, "ln1", lp=nc)
                    n1_views = [n1_bounce[hh][:].rearrange("(k p) t -> p k t",
                                                           p=128)
                                for hh in range(2)]
                    # batched applies: one big strided op per engine per half
                    # (DVE 6 tiles, Pool 2) instead of 32 per-tile ops
                    for hh in range(2):
                        k0 = 8 * hh
                        n1s = n1pool.tile([128, 8, CH], F8, tag="n1s",
                                          name="n1s")
                        tmpV = lnA.tile([128, 6, CH], BF16, tag="apV",
                                        name="tmpV")
                        nc.vector.tensor_mul(
                            tmpV[:], x_bf[:, k0:k0 + 6, :],
                            rstd_bf[:].unsqueeze(1).broadcast_to([128, 6, CH]))
                        nc.vector.tensor_add(
                            n1s[:, 0:6, :], tmpV[:],
                            shift_bf[:].unsqueeze(1).broadcast_to([128, 6, CH]))
                        tmpP = lnA.tile([128, 2, CH], BF16, tag="apP",
                                        name="tmpP")
                        nc.gpsimd.tensor_mul(
                            tmpP[:], x_bf[:, k0 + 6:k0 + 8, :],
                            rstd_bf[:].unsqueeze(1).broadcast_to([128, 2, CH]))
                        nc.gpsimd.tensor_add(
                            n1s[:, 6:8, :], tmpP[:],
                            shift_bf[:].unsqueeze(1).broadcast_to([128, 2, CH]))
                        nc.sync.dma_start(n1_views[hh][:], n1s[:])
                xbfpool.release()

                for hh in range(2):
                    if collectives:
                        nc.gpsimd.collective_compute(
                            "AllGather", OP.bypass, replica_groups=RG,
                            ins=[n1_bounce[hh].opt()], outs=[n1_full[hh].opt()])
                    else:  # timing variant: plain copy keeps the dependency edge
                        nc.sync.dma_start(n1_full[hh][0:C // 2, :],
                                          n1_bounce[hh][:])

                # ====== Phase B: QKV (all tokens, own 2 heads, fp8 DR) ======
                with tc.tile_pool(name="qkv_sb", bufs=1) as qkvp:
                    q_sb = qkvp.tile([128, 2, NTOK], BF16, tag="q_sb", name="q_sb")
                    k_sb = qkvp.tile([128, 2, NTOK], BF16, tag="k_sb", name="k_sb")
                    v_sb = qkvp.tile([128, NVT, 256], BF16, tag="v_sb", name="v_sb")

                    with tc.tile_pool(name="n1t", bufs=3) as n1tp, \
                         tc.tile_pool(name="psQK", bufs=1, space="PSUM") as psQK, \
                         tc.tile_pool(name="psV", bufs=1, space="PSUM") as psV:
                        nf_views = [n1_full[hh][:].rearrange(
                            "(r k p) t -> r p k t", r=N_CORES, p=128)
                            for hh in range(2)]
                        for r in range(N_CORES):
                            n1ca = n1tp.tile([128, NCT // 2, CH], F8, tag="n1ca",
                                             name="n1ca")
                            nc.sync.dma_start(n1ca[:], nf_views[0][r])
                            n1cb = n1tp.tile([128, NCT // 2, CH], F8, tag="n1cb",
                                             name="n1cb")
                            nc.sync.dma_start(n1cb[:], nf_views[1][r])
                            ps_q = [psQK.tile([128, CH], F32, tag=f"q{o}",
                                              name=f"ps_q{o}") for o in range(2)]
                            ps_k = [psQK.tile([128, CH], F32, tag=f"k{o}",
                                              name=f"ps_k{o}") for o in range(2)]
                            ps_v = [psV.tile([128, 256], F32, tag=f"v{s}",
                                             name=f"ps_v{s}") for s in range(S_SUB)]
                            for b in range(NB):
                                n1c = n1ca if b < NB // 2 else n1cb
                                bl = b % (NB // 2)
                                rhs = n1c[:, 2 * bl:2 * bl + 2, :]
                                st, sp = (b == 0), (b == NB - 1)
                                for o in range(2):
                                    nc.tensor.matmul(
                                        ps_q[o][:],
                                        wq_sb[:, (o * NB + b) * 2:(o * NB + b) * 2 + 2, :],
                                        rhs, start=st, stop=sp, perf_mode=PM.DoubleRow)
                                    nc.tensor.matmul(
                                        ps_k[o][:],
                                        wk_sb[:, (o * NB + b) * 2:(o * NB + b) * 2 + 2, :],
                                        rhs, start=st, stop=sp, perf_mode=PM.DoubleRow)
                                for s in range(S_SUB):
                                    nc.tensor.matmul(
                                        ps_v[s][:],
                                        n1c[:, 2 * bl:2 * bl + 2, 128 * s:128 * (s + 1)],
                                        wv_sb[:, 2 * b:2 * b + 2, :],
                                        start=st, stop=sp, perf_mode=PM.DoubleRow)
                            for o in range(2):
                                nc.vector.tensor_scalar(
                                    q_sb[:, o, CH * r:CH * (r + 1)], ps_q[o][:],
                                    RWS, bq_sb[:, o, :], op0=OP.mult, op1=OP.add)
                                nc.vector.tensor_scalar(
                                    k_sb[:, o, CH * r:CH * (r + 1)], ps_k[o][:],
                                    RWS, bk_sb[:, o, :], op0=OP.mult, op1=OP.add)
                            for s in range(S_SUB):
                                nc.vector.scalar_tensor_tensor(
                                    v_sb[:, r * S_SUB + s, :], ps_v[s][:], RWS,
                                    bv_sb[:], op0=OP.mult, op1=OP.add)

                    # ========= Phase B2: attention per (head, batch) =========
                    WPC = NCT * NB * 2 // 8
                    wp_pieces = iter(range(8))
                    with tc.tile_pool(name="attn_e", bufs=4) as ep, \
                         tc.tile_pool(name="attn_acc", bufs=2) as accp, \
                         tc.tile_pool(name="attn_small", bufs=3) as asml, \
                         tc.tile_pool(name="vals", bufs=3) as valsp, \
                         tc.tile_pool(name="psS", bufs=3, space="PSUM") as psS, \
                         tc.tile_pool(name="psAV", bufs=1, space="PSUM") as psAV, \
                         tc.tile_pool(name="psDen", bufs=1, space="PSUM") as psDen:
                        for h in range(H_PER_CORE):
                            for bb in range(B):
                                for j in range(NQS):
                                    # stream the proj weights behind the n1c
                                    # loads, spread so no critical transfer is
                                    # ever stuck behind a long one
                                    pc = next(wp_pieces, None)
                                    if pc is not None:
                                        nc.sync.dma_start(
                                            wp_sb[:, WPC * pc:WPC * (pc + 1), :],
                                            wproj8[:, WPC * pc:WPC * (pc + 1), :])
                                    ni = 4 * (j + 1) if QS == 512 else T // 128
                                    ps_av = psAV.tile([128, QS], F32, tag="av",
                                                      name="ps_av")
                                    ps_den = psDen.tile([128, QS], F32, tag="den",
                                                        name="ps_den")
                                    e_acc = accp.tile([128, QS], BF16, tag="eacc",
                                                      name="e_acc")
                                    qtok = bb * T + j * QS
                                    for u in range(ni // 2):
                                        # paired score tiles share one Exp call
                                        # over [128, 1024] (amortizes Act setup)
                                        ps_s2 = psS.tile([128, 2, QS], F32,
                                                         tag="s2", name="ps_s2")
                                        for hf in range(2):
                                            i = 2 * u + hf
                                            ktok = bb * T + i * 128
                                            nc.tensor.matmul(
                                                ps_s2[:, hf, :],
                                                k_sb[:, h, ktok:ktok + 128],
                                                q_sb[:, h, qtok:qtok + QS],
                                                start=True, stop=True)
                                        e2 = ep.tile([128, 2, QS], BF16, tag="e2",
                                                     name="e2")
                                        nc.scalar.activation(e2[:], ps_s2[:],
                                                             AF.Exp, bias=0.0,
                                                             scale=SCALE)
                                        d0 = 2 * u - (ni - 4)
                                        if d0 >= 0:
                                            nc.vector.tensor_mul(
                                                e2[:], e2[:],
                                                masks_sb[:, d0:d0 + 2, :])
                                        for hf in range(2):
                                            i = 2 * u + hf
                                            # hybrid denominator: diagonal tiles
                                            # accumulate on PE, the rest on DVE
                                            if i < ni - 4:
                                                if i == 0:
                                                    nc.vector.tensor_copy(
                                                        e_acc[:], e2[:, hf, :])
                                                else:
                                                    nc.vector.tensor_add(
                                                        e_acc[:], e_acc[:],
                                                        e2[:, hf, :])
                                            else:
                                                nc.tensor.matmul(
                                                    ps_den[:], ones_bf[:],
                                                    e2[:, hf, :],
                                                    start=(i == ni - 4),
                                                    stop=(i == ni - 1 and ni == 4))
                                            tt = (bb * T + i * 128) // 128
                                            nc.tensor.matmul(
                                                ps_av[:],
                                                v_sb[:, tt, 128 * h:128 * (h + 1)],
                                                e2[:, hf, :],
                                                start=(i == 0), stop=(i == ni - 1))
                                    if ni > 4:
                                        nc.tensor.matmul(ps_den[:], ones_bf[:],
                                                         e_acc[:], start=False,
                                                         stop=True)
                                    rec_bc = asml.tile([128, QS], F32, tag="rec",
                                                       name="rec_bc")
                                    nc.vector.reciprocal(rec_bc[:], ps_den[:])
                                    vtile = valsp.tile([128, QS], F8, tag="vt",
                                                       name="vtile")
                                    nc.vector.tensor_mul(vtile[:], ps_av[:],
                                                         rec_bc[:])
                                    ncol0 = bb * T + j * QS
                                    for part in range(max(1, QS // CH)):
                                        jg = (ncol0 + part * CH) // CH
                                        w = min(CH, QS)
                                        nc.sync.dma_start(
                                            a2a_in[h][128 * jg:128 * (jg + 1), :],
                                            vtile[:, part * w:(part + 1) * w])
                            if h == 0:
                                # h=0 values complete at half-time: overlap the
                                # first AllToAll with the h=1 attention pass
                                if collectives:
                                    nc.gpsimd.collective_compute(
                                        "AllToAll", OP.bypass, replica_groups=RG,
                                        ins=[a2a_in[0].opt()],
                                        outs=[a2a_out[0].opt()])
                                else:
                                    nc.sync.dma_start(a2a_out[0][:], a2a_in[0][:])

                wqkvp.release()
                if collectives:
                    nc.gpsimd.collective_compute(
                        "AllToAll", OP.bypass, replica_groups=RG,
                        ins=[a2a_in[1].opt()], outs=[a2a_out[1].opt()])
                else:
                    nc.sync.dma_start(a2a_out[1][:], a2a_in[1][:])

                # ====== Phase C: proj + residual + LN2 stats (own chunk) ======
                with tc.tile_pool(name="vf", bufs=1) as vfp, \
                     tc.tile_pool(name="pdrain", bufs=3) as pdp, \
                     tc.tile_pool(name="lnC_small", bufs=1) as lnCs, \
                     tc.tile_pool(name="psP", bufs=3, space="PSUM") as psP, \
                     tc.tile_pool(name="psP2", bufs=1, space="PSUM") as psP2:
                    vf_sb = vfp.tile([128, NB, 2, CH], F8, tag="vf",
                                     name="vf_sb")
                    for hs in range(2):
                        nc.sync.dma_start(
                            vf_sb[:, :, hs, :],
                            a2a_out[hs][:].rearrange("(r p) t -> p r t", p=128))
                    ps_sum2 = psP2.tile([128, CH], F32, tag="sum2", name="ps_sum2")
                    ps_ssq2 = psP2.tile([128, CH], F32, tag="ssq2", name="ps_ssq2")
                    r1bfp = tc.alloc_tile_pool(name="r1bf", bufs=1)
                    r1_bf = r1bfp.tile([128, NCT, CH], BF16, tag="r1_bf",
                                       name="r1_bf")

                    def ln2_stats(ot):
                        # lag-2 interleave behind the proj loop: r1[ot] is ready
                        # two iterations later, so the PE stats matmuls never
                        # stall on the drain chain
                        if ot % 2 == 0:
                            nc.scalar.activation(r1_bf[:, ot, :], r1_sb[:, ot, :],
                                                 AF.Identity)
                        else:
                            nc.vector.tensor_copy(r1_bf[:, ot, :],
                                                  r1_sb[:, ot, :])
                        sq2 = pdp.tile([128, CH], BF16, tag="sq2", name="sq2")
                        nc.vector.tensor_mul(sq2[:], r1_bf[:, ot, :],
                                             r1_bf[:, ot, :])
                        nc.tensor.matmul(ps_sum2[:], ones_bf[:], r1_bf[:, ot, :],
                                         start=(ot == 0), stop=(ot == NCT - 1))
                        nc.tensor.matmul(ps_ssq2[:], ones_bf[:], sq2[:],
                                         start=(ot == 0), stop=(ot == NCT - 1))

                    for ot in range(NCT):
                        ps_p = psP.tile([128, CH], F32, tag="p", name="ps_p")
                        for b in range(NB):
                            nc.tensor.matmul(
                                ps_p[:],
                                wp_sb[:, (ot * NB + b) * 2:(ot * NB + b) * 2 + 2, :],
                                vf_sb[:, b, :, :],
                                start=(b == 0), stop=(b == NB - 1),
                                perf_mode=PM.DoubleRow)
                        p_t = pdp.tile([128, CH], F32, tag="p_t", name="p_t")
                        nc.scalar.activation(p_t[:], ps_p[:], AF.Identity,
                                             bias=bproj_sb[:, ot, :], scale=RWS)
                        nc.vector.tensor_add(r1_sb[:, ot, :], p_t[:],
                                             x_tile(ot))
                        if ot >= 2:
                            ln2_stats(ot - 2)
                    ln2_stats(NCT - 2)
                    ln2_stats(NCT - 1)
                    rstd2_bf, shift2_bf = _ln_finish(nc, lnCs, ps_sum2, ps_ssq2,
                                                     C, CH, "ln2", lp=nc)
                    tmp2V = pdp.tile([128, 11, CH], BF16, tag="ap2V",
                                     name="tmp2V")
                    nc.vector.tensor_mul(
                        tmp2V[:], r1_bf[:, 0:11, :],
                        rstd2_bf[:].unsqueeze(1).broadcast_to([128, 11, CH]))
                    nc.vector.tensor_add(
                        n2_sb[:, 0:11, :], tmp2V[:],
                        shift2_bf[:].unsqueeze(1).broadcast_to([128, 11, CH]))
                    tmp2P = pdp.tile([128, 5, CH], BF16, tag="ap2P",
                                     name="tmp2P")
                    nc.gpsimd.tensor_mul(
                        tmp2P[:], r1_bf[:, 11:16, :],
                        rstd2_bf[:].unsqueeze(1).broadcast_to([128, 5, CH]))
                    nc.gpsimd.tensor_add(
                        n2_sb[:, 11:16, :], tmp2P[:],
                        shift2_bf[:].unsqueeze(1).broadcast_to([128, 5, CH]))
                    r1bfp.release()

                projw.release()
                # =============== Phase D: FFN1 (fp8 DR) ===============
                with tc.tile_pool(name="hpool", bufs=1) as hpool, \
                     tc.tile_pool(name="w1", bufs=6) as w1p, \
                     tc.tile_pool(name="psH", bufs=3, space="PSUM") as psH:
                    h_sb = hpool.tile([128, NFT, CH], F8, tag="h_sb",
                                      name="h_sb")
                    for ft in range(NFT):
                        w1t = w1p.tile([128, NB * 2, 128], F8, tag="w1",
                                       name="w1t")
                        nc.sync.dma_start(w1t[:], wf18[ft])
                        ps_h = psH.tile([128, CH], F32, tag="h", name="ps_h")
                        for b in range(NB):
                            nc.tensor.matmul(
                                ps_h[:], w1t[:, 2 * b:2 * b + 2, :],
                                n2_sb[:, 2 * b:2 * b + 2, :],
                                start=(b == 0), stop=(b == NB - 1),
                                perf_mode=PM.DoubleRow)
                        nc.scalar.activation(h_sb[:, ft, :], ps_h[:], GELU,
                                             bias=bf1_sb[:, ft, :], scale=RWS)

                    # ============= Phase D2: FFN2 (fp8 DR) =============
                    with tc.tile_pool(name="w2", bufs=3) as w2p, \
                         tc.tile_pool(name="outp", bufs=3) as outp, \
                         tc.tile_pool(name="psF", bufs=2, space="PSUM") as psF:
                        for ot in range(NCT):
                            w2t = w2p.tile([128, NFB * 2, 128], F8, tag="w2",
                                           name="w2t")
                            nc.sync.dma_start(w2t[:], wf28[ot])
                            ps_f = psF.tile([128, CH], F32, tag="f",
                                            name="ps_f")
                            for g in range(NFB):
                                nc.tensor.matmul(
                                    ps_f[:], w2t[:, 2 * g:2 * g + 2, :],
                                    h_sb[:, 2 * g:2 * g + 2, :],
                                    start=(g == 0), stop=(g == NFB - 1),
                                    perf_mode=PM.DoubleRow)
                            f_t = outp.tile([128, CH], F32, tag="f_t",
                                            name="f_t")
                            nc.scalar.activation(f_t[:], ps_f[:], AF.Identity,
                                                 bias=bf2_sb[:, ot, :],
                                                 scale=RWS)
                            o_t = outp.tile([128, CH], F32, tag="o_t",
                                            name="o_t")
                            nc.vector.tensor_add(o_t[:], f_t[:],
                                                 r1_sb[:, ot, :])
                            nc.sync.dma_start(out[128 * ot:128 * (ot + 1), :],
                                              o_t[:])
                n2pool.release()

    nc.compile()
    return nc


# ----------------------------------------------------------------------------
# Host side
# ----------------------------------------------------------------------------

_NC_CACHE = {}


def _get_nc(T=2048):
    if T not in _NC_CACHE:
        _NC_CACHE[T] = build_decoder(T)
    return _NC_CACHE[T]


def _q8(a):
    """Quantize f32 -> fp8 e4m3 bytes with the x32 pre-scale."""
    return (np.ascontiguousarray(a, np.float32) * WS).astype(
        ml_dtypes.float8_e4m3).view(np.uint8)


def _dr_lhsT_flat(W):
    """[K, M] f32 -> DoubleRow lhsT SBUF layout [128, (M/128 * K/256 * 2), 128]."""
    K, M = W.shape
    a = W.reshape(K // 256, 2, 128, M // 128, 128).transpose(2, 3, 0, 1, 4)
    return np.ascontiguousarray(a.reshape(128, (M // 128) * (K // 256) * 2, 128))


def _dr_lhsT_tiles(W):
    """[K, M] f32 -> per-out-tile DoubleRow layout [M/128, 128, K/256 * 2, 128]."""
    K, M = W.shape
    a = W.reshape(K // 256, 2, 128, M // 128, 128).transpose(3, 2, 0, 1, 4)
    return np.ascontiguousarray(a.reshape(M // 128, 128, (K // 256) * 2, 128))


def _bf16(a):
    return np.ascontiguousarray(a).astype(ml_dtypes.bfloat16).view(np.uint16)


def _prep_inputs(x, Wqkv, bqkv, Wproj, bproj, Wf1, bf1, Wf2, bf2,
                 g1, b1, g2, b2):
    """Fold LN affines, slice heads per core, build per-core in_maps."""
    f32 = np.float32
    x = np.asarray(x, f32)
    Bx, T, Cx = x.shape
    NTOK = Bx * T
    CH = NTOK // N_CORES
    Wqkv = np.asarray(Wqkv, f32)
    bqkv = np.asarray(bqkv, f32)
    g1 = np.asarray(g1, f32); b1 = np.asarray(b1, f32)
    g2 = np.asarray(g2, f32); b2 = np.asarray(b2, f32)
    Wqkv_eff = g1[:, None] * Wqkv
    bqkv_eff = b1 @ Wqkv + bqkv
    Wf1 = np.asarray(Wf1, f32)
    bf1v = np.asarray(bf1, f32)
    Wf1_eff = g2[:, None] * Wf1
    bf1_eff = b2 @ Wf1 + bf1v
    Wproj = np.asarray(Wproj, f32)
    bprojv = np.asarray(bproj, f32)
    Wf2 = np.asarray(Wf2, f32)
    bf2v = np.asarray(bf2, f32)

    xt = np.ascontiguousarray(x.reshape(NTOK, Cx).T)        # [C, NT]

    QS = min(512, T)
    masks = np.zeros((128, 4, QS), f32)
    p = np.arange(128)[:, None]
    fcol = np.arange(QS)[None, :]
    for m in range(4):
        masks[:, m, :] = (p <= fcol - 128 * m).astype(f32)

    shared = {
        "wproj8": _q8(_dr_lhsT_flat(Wproj)),
        "bproj": np.ascontiguousarray(
            bprojv.reshape(Cx // 128, 128).T.reshape(128, Cx // 128, 1)),
        "wf18": _q8(_dr_lhsT_tiles(Wf1_eff)),
        "bf1": np.ascontiguousarray(
            bf1_eff.reshape(F // 128, 128).T.reshape(128, F // 128, 1)),
        "wf28": _q8(_dr_lhsT_tiles(Wf2)),
        "bf2": np.ascontiguousarray(
            bf2v.reshape(Cx // 128, 128).T.reshape(128, Cx // 128, 1)),
        "masks": _bf16(masks),
    }
    in_maps = []
    for c in range(N_CORES):
        h0, h1 = 2 * c, 2 * c + 1
        qcols = np.concatenate([h0 * 384 + np.arange(128),
                                h1 * 384 + np.arange(128)])
        kcols = qcols + 128
        vcols = qcols + 256
        m = dict(shared)
        m["xt"] = np.ascontiguousarray(xt[:, c * CH:(c + 1) * CH])
        m["wq8"] = _q8(_dr_lhsT_flat(Wqkv_eff[:, qcols]))
        m["wk8"] = _q8(_dr_lhsT_flat(Wqkv_eff[:, kcols]))
        # v rhs SBUF layout: [128, K/256 * 2, 256]
        wv = Wqkv_eff[:, vcols].reshape(8, 2, 128, 256).transpose(2, 0, 1, 3)
        m["wv8"] = _q8(np.ascontiguousarray(wv.reshape(128, 16, 256)))
        m["bq"] = np.ascontiguousarray(
            bqkv_eff[qcols].reshape(2, 128).T.reshape(128, 2, 1))
        m["bk"] = np.ascontiguousarray(
            bqkv_eff[kcols].reshape(2, 128).T.reshape(128, 2, 1))
        m["bv_bc"] = np.ascontiguousarray(
            np.broadcast_to(bqkv_eff[vcols][None, :], (128, 256)))
        in_maps.append(m)
    return in_maps, (Bx, T, Cx, CH)


def kernel(x, Wqkv, bqkv, Wproj, bproj, Wf1, bf1, Wf2, bf2,
           g1, b1, g2, b2, _trace=False):
    in_maps, (Bx, T, Cx, CH) = _prep_inputs(
        x, Wqkv, bqkv, Wproj, bproj, Wf1, bf1, Wf2, bf2, g1, b1, g2, b2)
    nc = _get_nc(T)
    res = bass_utils.run_bass_kernel_spmd(
        nc, in_maps, core_ids=list(range(N_CORES)), trace=_trace)
    kernel.last_results = res
    NTOK = Bx * T
    out_t = np.empty((NTOK, Cx), np.float32)
    for c in range(N_CORES):
        out_t[c * CH:(c + 1) * CH, :] = res.results[c]["out"].T
    return out_t.reshape(Bx, T, Cx)


# revision 21
# speedup vs baseline: 1.0412x; 1.0412x over previous
"""Trainium2 Bass kernel for a dense decoder block (LN->MHA->res, LN->FFN->res).

Sharding (8 cores, one NEFF, SPMD-uniform addressing):
  - LN1 token-parallel (512-token chunk/core) -> AllGather of normalized acts
    quantized to fp8e4m3 (x32-scaled weights keep fp8 out of subnormals).
  - QKV + attention head-parallel (2 heads/core, causal, unstable softmax --
    exact because masked logits multiply to 0 post-exp).
  - AllToAll redistributes attention values (fp8): head-shards -> token-shards.
  - proj + residual + LN2 + FFN token-parallel with fp8 weights streamed.
  - LN affine params are folded into the following matmul weights on host.

All heavy GEMMs run in fp8e4m3 with MatmulPerfMode.DoubleRow (256-row
contraction per instruction, 0.5 cycles/output column -> 4x f32r MACs/cycle).
Weights are pre-scaled by 32 on host; PSUM drains apply 1/32 via the
activation-engine scale. Attention (scores/softmax/AV) runs in bf16.
LN statistics use the ones-matmul trick in f32r (1 cycle/row).
Activations stay channel-major [C, tokens]; v is produced token-major
directly by swapping matmul operands, so no transposes are needed.
"""

import math

import numpy as np
import ml_dtypes

import concourse.bass as bass
import concourse.mybir as mybir
import concourse.tile as tile
from concourse import bacc
from concourse import bass_utils

F32 = mybir.dt.float32
F32R = mybir.dt.float32r
BF16 = mybir.dt.bfloat16
F8 = mybir.dt.float8e4
AF = mybir.ActivationFunctionType
OP = mybir.AluOpType
PM = mybir.MatmulPerfMode

N_CORES = 8
B = 2
C = 2048
H = 16
HD = 128
F = 8192
NT = B * 2048                       # total tokens (B*T with T=2048)
H_PER_CORE = H // N_CORES           # 2
NCT = C // 128                      # 16 channel tiles
NB = C // 256                       # 8 DoubleRow contraction blocks
NFT = F // 128                      # 64 ffn tiles
NFB = F // 256                      # 32 ffn DoubleRow blocks
EPS = 1e-5
SCALE = 1.0 / math.sqrt(HD)
WS = 32.0                           # host-side weight scale (drains apply 1/WS)
RWS = 1.0 / WS
GELU = AF.Gelu_apprx_tanh


def r32(ap):
    return ap.bitcast(F32R)


def _ln_finish(nc, pool_small, ps_sum, ps_ssq, n_tok, ncols, tagpfx, lp=None):
    """From broadcast sum/sumsq psums produce SBUF rstd/shift [128, ncols].

    With lp set, outputs are bf16 (feeding the bf16 apply path)."""
    odt = BF16 if lp is not None else F32
    mean = pool_small.tile([128, ncols], F32, tag=f"{tagpfx}_mean", name="mean")
    nc.vector.tensor_scalar_mul(mean[:], ps_sum[:], 1.0 / n_tok)
    # msq = (sum/n)^2 on Act, in parallel with the DVE mean
    msq = pool_small.tile([128, ncols], F32, tag=f"{tagpfx}_msq", name="msq")
    nc.scalar.activation(msq[:], ps_sum[:], AF.Square, scale=1.0 / n_tok)
    varp0 = pool_small.tile([128, ncols], F32, tag=f"{tagpfx}_varp0",
                            name="varp0")
    nc.vector.tensor_scalar(varp0[:], ps_ssq[:], 1.0 / n_tok, EPS,
                            op0=OP.mult, op1=OP.add)
    varp = pool_small.tile([128, ncols], F32, tag=f"{tagpfx}_varp", name="varp")
    nc.vector.scalar_tensor_tensor(varp[:], msq[:], -1.0, varp0[:],
                                   op0=OP.mult, op1=OP.add)
    std = pool_small.tile([128, ncols], F32, tag=f"{tagpfx}_std", name="std")
    nc.scalar.sqrt(std[:], varp[:])
    rstd_bc = pool_small.tile([128, ncols], odt, tag=f"{tagpfx}_rstd", name="rstd")
    if lp is not None:
        with lp.allow_low_precision(reason="rstd broadcast feeds fp8 path"):
            nc.vector.reciprocal(rstd_bc[:], std[:])
    else:
        nc.vector.reciprocal(rstd_bc[:], std[:])
    shift_bc = pool_small.tile([128, ncols], odt, tag=f"{tagpfx}_shift", name="shift")
    nc.vector.scalar_tensor_tensor(shift_bc[:], mean[:], -1.0, rstd_bc[:],
                                   op0=OP.mult, op1=OP.mult)
    return rstd_bc, shift_bc


def build_decoder(T=2048, collectives=True):
    """Build the SPMD decoder-block program for seq length T (2048 = real)."""
    NTOK = B * T
    CH = NTOK // N_CORES            # tokens per core chunk (512)
    NQS = max(1, T // 512)          # q slices of 512 per batch elem
    QS = min(512, T)
    NVT = NTOK // 128               # token-major v tiles (32)
    S_SUB = CH // 128               # 128-token subtiles per chunk (4)

    nc = bacc.Bacc("TRN2", target_bir_lowering=False, debug=False,
                   num_devices=N_CORES)

    # ---- I/O ----
    xt = nc.dram_tensor("xt", [C, CH], F32, kind="ExternalInput").ap()
    wq8 = nc.dram_tensor("wq8", [128, 2 * NB * 2, 128], F8, kind="ExternalInput").ap()
    wk8 = nc.dram_tensor("wk8", [128, 2 * NB * 2, 128], F8, kind="ExternalInput").ap()
    wv8 = nc.dram_tensor("wv8", [128, NB * 2, 256], F8, kind="ExternalInput").ap()
    bq = nc.dram_tensor("bq", [128, 2, 1], F32, kind="ExternalInput").ap()
    bk = nc.dram_tensor("bk", [128, 2, 1], F32, kind="ExternalInput").ap()
    bv_bc = nc.dram_tensor("bv_bc", [128, 256], F32, kind="ExternalInput").ap()
    wproj8 = nc.dram_tensor("wproj8", [128, NCT * NB * 2, 128], F8,
                            kind="ExternalInput").ap()
    bproj = nc.dram_tensor("bproj", [128, NCT, 1], F32, kind="ExternalInput").ap()
    wf18 = nc.dram_tensor("wf18", [NFT, 128, NB * 2, 128], F8,
                          kind="ExternalInput").ap()
    bf1 = nc.dram_tensor("bf1", [128, NFT, 1], F32, kind="ExternalInput").ap()
    wf28 = nc.dram_tensor("wf28", [NCT, 128, NFB * 2, 128], F8,
                          kind="ExternalInput").ap()
    bf2 = nc.dram_tensor("bf2", [128, NCT, 1], F32, kind="ExternalInput").ap()
    masks = nc.dram_tensor("masks", [128, 4, QS], BF16, kind="ExternalInput").ap()
    out = nc.dram_tensor("out", [C, CH], F32, kind="ExternalOutput").ap()

    RG = [list(range(N_CORES))]

    with tile.TileContext(nc) as tc:
        with tc.tile_pool(name="dram", bufs=1, space="DRAM") as dram, \
             tc.tile_pool(name="persist", bufs=1) as persist:
            n1_bounce = [dram.tile([C // 2, CH], F8, tag=f"n1_bounce{hh}",
                                   name="n1_bounce") for hh in range(2)]
            n1_full = [dram.tile([N_CORES * C // 2, CH], F8, tag=f"n1_full{hh}",
                                 name="n1_full", addr_space="Shared")
                       for hh in range(2)]
            a2a_in = [dram.tile([C // 2, CH], F8, tag=f"a2a_in{h}",
                                name="a2a_in") for h in range(2)]
            a2a_out = [dram.tile([C // 2, CH], F8, tag=f"a2a_out{h}",
                                 name="a2a_out") for h in range(2)]

            # x tiles are the critical path at t=0: issue their DMAs first.
            xt_view = xt.rearrange("(k p) t -> p k t", p=128)
            ones_sq = persist.tile([128, 128], F32, tag="ones_sq", name="ones_sq")
            ones_bf = persist.tile([128, 128], BF16, tag="ones_bf", name="ones_bf")
            nc.vector.memset(ones_sq[:], 1.0)
            nc.vector.tensor_copy(ones_bf[:], ones_sq[:])
            masks_sb = persist.tile([128, 4, QS], BF16, tag="masks", name="masks_sb")
            bq_sb = persist.tile([128, 2, 1], F32, tag="bq", name="bq_sb")
            bk_sb = persist.tile([128, 2, 1], F32, tag="bk", name="bk_sb")
            bv_sb = persist.tile([128, 256], F32, tag="bv", name="bv_sb")
            bproj_sb = persist.tile([128, NCT, 1], F32, tag="bproj", name="bproj_sb")
            bf1_sb = persist.tile([128, NFT, 1], F32, tag="bf1", name="bf1_sb")
            bf2_sb = persist.tile([128, NCT, 1], F32, tag="bf2", name="bf2_sb")

            # r1 survives proj -> final residual add; x survives LN1 -> proj.
            r1_sb = persist.tile([128, NCT, CH], F32, tag="r1", name="r1_sb")

            with tc.tile_pool(name="xpool", bufs=1) as xpool:
                # four separate tiles: a single x tile would serialize each
                # quarter's DMA behind the previous quarter's readers (the
                # dependency tracker is tile-granular)
                x_q = [xpool.tile([128, 4, CH], F32, tag=f"x_q{q}", name="x_q")
                       for q in range(4)]

                def x_tile(k):
                    return x_q[k // 4][:, k % 4, :]
                n2pool = tc.alloc_tile_pool(name="n2pool", bufs=1)
                n2_sb = n2pool.tile([128, NCT, CH], F8, tag="n2_sb",
                                    name="n2_sb")
                projw = tc.alloc_tile_pool(name="projw", bufs=1)
                wp_sb = projw.tile([128, NCT * NB * 2, 128], F8, tag="wp",
                                   name="wp_sb")
                wqkvp = tc.alloc_tile_pool(name="wqkv", bufs=1)
                wq_sb = wqkvp.tile([128, 2 * NB * 2, 128], F8, tag="wq",
                                   name="wq_sb")
                wk_sb = wqkvp.tile([128, 2 * NB * 2, 128], F8, tag="wk",
                                   name="wk_sb")
                wv_sb = wqkvp.tile([128, NB * 2, 256], F8, tag="wv",
                                   name="wv_sb")
                xbfpool = tc.alloc_tile_pool(name="xbfpool", bufs=1)
                x_bf = xbfpool.tile([128, NCT, CH], BF16, tag="x_bf", name="x_bf")

                # ================= Phase A: LN1 on own chunk =================
                with tc.tile_pool(name="lnA", bufs=3) as lnA, \
                     tc.tile_pool(name="lnA_small", bufs=1) as lnAs, \
                     tc.tile_pool(name="n1pool", bufs=2) as n1pool, \
                     tc.tile_pool(name="psA", bufs=1, space="PSUM") as psA:
                    ps_sum = psA.tile([128, CH], F32, tag="sum", name="ps_sum")
                    ps_ssq = psA.tile([128, CH], F32, tag="ssq", name="ps_ssq")
                    for q in range(4):
                        for hq in range(2):
                            nc.sync.dma_start(
                                x_q[q][:, 2 * hq:2 * hq + 2, :],
                                xt_view[:, 4 * q + 2 * hq:4 * q + 2 * hq + 2, :])
                    nc.sync.dma_start(wq_sb[:], wq8)
                    nc.sync.dma_start(wk_sb[:], wk8)
                    nc.sync.dma_start(wv_sb[:], wv8)
                    nc.sync.dma_start(bq_sb[:], bq)
                    nc.sync.dma_start(bk_sb[:], bk)
                    nc.sync.dma_start(bv_sb[:], bv_bc)
                    nc.sync.dma_start(masks_sb[:], masks)
                    nc.sync.dma_start(bproj_sb[:], bproj)
                    nc.sync.dma_start(bf1_sb[:], bf1)
                    nc.sync.dma_start(bf2_sb[:], bf2)
                    for k in range(NCT):
                        nc.scalar.activation(x_bf[:, k, :], x_tile(k),
                                             AF.Identity)
                        sq = lnA.tile([128, CH], BF16, tag="sq", name="sq")
                        nc.vector.tensor_mul(sq[:], x_bf[:, k, :], x_bf[:, k, :])
                        nc.tensor.matmul(ps_sum[:], ones_bf[:], x_bf[:, k, :],
                                         start=(k == 0), stop=(k == NCT - 1))
                        nc.tensor.matmul(ps_ssq[:], ones_bf[:], sq[:],
                                         start=(k == 0), stop=(k == NCT - 1))
                    rstd_bf, shift_bf = _ln_finish(nc, lnAs, ps_sum, ps_ssq,
                                                   C, CH, "ln1", lp=nc)
                    n1_views = [n1_bounce[hh][:].rearrange("(k p) t -> p k t",
                                                           p=128)
                                for hh in range(2)]
                    # batched applies: one big strided op per engine per half
                    # (DVE 6 tiles, Pool 2) instead of 32 per-tile ops
                    for hh in range(2):
                        k0 = 8 * hh
                        n1s = n1pool.tile([128, 8, CH], F8, tag="n1s",
                                          name="n1s")
                        tmpV = lnA.tile([128, 6, CH], BF16, tag="apV",
                                        name="tmpV")
                        nc.vector.tensor_mul(
                            tmpV[:], x_bf[:, k0:k0 + 6, :],
                            rstd_bf[:].unsqueeze(1).broadcast_to([128, 6, CH]))
                        nc.vector.tensor_add(
                            n1s[:, 0:6, :], tmpV[:],
                            shift_bf[:].unsqueeze(1).broadcast_to([128, 6, CH]))
                        tmpP = lnA.tile([128, 2, CH], BF16, tag="apP",
                                        name="tmpP")
                        nc.gpsimd.tensor_mul(
                            tmpP[:], x_bf[:, k0 + 6:k0 + 8, :],
                            rstd_bf[:].unsqueeze(1).broadcast_to([128, 2, CH]))
                        nc.gpsimd.tensor_add(
                            n1s[:, 6:8, :], tmpP[:],
                            shift_bf[:].unsqueeze(1).broadcast_to([128, 2, CH]))
                        nc.sync.dma_start(n1_views[hh][:], n1s[:])
                xbfpool.release()

                for hh in range(2):
                    if collectives:
                        nc.gpsimd.collective_compute(
                            "AllGather", OP.bypass, replica_groups=RG,
                            ins=[n1_bounce[hh].opt()], outs=[n1_full[hh].opt()])
                    else:  # timing variant: plain copy keeps the dependency edge
                        nc.sync.dma_start(n1_full[hh][0:C // 2, :],
                                          n1_bounce[hh][:])

                # ====== Phase B: QKV (all tokens, own 2 heads, fp8 DR) ======
                with tc.tile_pool(name="qkv_sb", bufs=1) as qkvp:
                    q_sb = qkvp.tile([128, 2, NTOK], BF16, tag="q_sb", name="q_sb")
                    k_sb = qkvp.tile([128, 2, NTOK], BF16, tag="k_sb", name="k_sb")
                    v_sb = qkvp.tile([128, NVT, 256], BF16, tag="v_sb", name="v_sb")

                    with tc.tile_pool(name="n1t", bufs=3) as n1tp, \
                         tc.tile_pool(name="psQK", bufs=1, space="PSUM") as psQK, \
                         tc.tile_pool(name="psV", bufs=1, space="PSUM") as psV:
                        nf_views = [n1_full[hh][:].rearrange(
                            "(r k p) t -> r p k t", r=N_CORES, p=128)
                            for hh in range(2)]
                        for r in range(N_CORES):
                            n1ca = n1tp.tile([128, NCT // 2, CH], F8, tag="n1ca",
                                             name="n1ca")
                            nc.sync.dma_start(n1ca[:], nf_views[0][r])
                            n1cb = n1tp.tile([128, NCT // 2, CH], F8, tag="n1cb",
                                             name="n1cb")
                            nc.sync.dma_start(n1cb[:], nf_views[1][r])
                            ps_q = [psQK.tile([128, CH], F32, tag=f"q{o}",
                                              name=f"ps_q{o}") for o in range(2)]
                            ps_k = [psQK.tile([128, CH], F32, tag=f"k{o}",
                                              name=f"ps_k{o}") for o in range(2)]
                            ps_v = [psV.tile([128, 256], F32, tag=f"v{s}",
                                             name=f"ps_v{s}") for s in range(S_SUB)]
                            for b in range(NB):
                                n1c = n1ca if b < NB // 2 else n1cb
                                bl = b % (NB // 2)
                                rhs = n1c[:, 2 * bl:2 * bl + 2, :]
                                st, sp = (b == 0), (b == NB - 1)
                                for o in range(2):
                                    nc.tensor.matmul(
                                        ps_q[o][:],
                                        wq_sb[:, (o * NB + b) * 2:(o * NB + b) * 2 + 2, :],
                                        rhs, start=st, stop=sp, perf_mode=PM.DoubleRow)
                                    nc.tensor.matmul(
                                        ps_k[o][:],
                                        wk_sb[:, (o * NB + b) * 2:(o * NB + b) * 2 + 2, :],
                                        rhs, start=st, stop=sp, perf_mode=PM.DoubleRow)
                                for s in range(S_SUB):
                                    nc.tensor.matmul(
                                        ps_v[s][:],
                                        n1c[:, 2 * bl:2 * bl + 2, 128 * s:128 * (s + 1)],
                                        wv_sb[:, 2 * b:2 * b + 2, :],
                                        start=st, stop=sp, perf_mode=PM.DoubleRow)
                            for o in range(2):
                                nc.vector.tensor_scalar(
                                    q_sb[:, o, CH * r:CH * (r + 1)], ps_q[o][:],
                                    RWS, bq_sb[:, o, :], op0=OP.mult, op1=OP.add)
                                nc.vector.tensor_scalar(
                                    k_sb[:, o, CH * r:CH * (r + 1)], ps_k[o][:],
                                    RWS, bk_sb[:, o, :], op0=OP.mult, op1=OP.add)
                            for s in range(S_SUB):
                                nc.vector.scalar_tensor_tensor(
                                    v_sb[:, r * S_SUB + s, :], ps_v[s][:], RWS,
                                    bv_sb[:], op0=OP.mult, op1=OP.add)

                    # ========= Phase B2: attention per (head, batch) =========
                    WPC = NCT * NB * 2 // 8
                    wp_pieces = iter(range(8))
                    with tc.tile_pool(name="attn_e", bufs=4) as ep, \
                         tc.tile_pool(name="attn_acc", bufs=2) as accp, \
                         tc.tile_pool(name="attn_small", bufs=3) as asml, \
                         tc.tile_pool(name="vals", bufs=3) as valsp, \
                         tc.tile_pool(name="psS", bufs=3, space="PSUM") as psS, \
                         tc.tile_pool(name="psAV", bufs=1, space="PSUM") as psAV, \
                         tc.tile_pool(name="psDen", bufs=1, space="PSUM") as psDen:
                        for h in range(H_PER_CORE):
                            for bb in range(B):
                                for j in range(NQS):
                                    # stream the proj weights behind the n1c
                                    # loads, spread so no critical transfer is
                                    # ever stuck behind a long one
                                    pc = next(wp_pieces, None)
                                    if pc is not None:
                                        nc.sync.dma_start(
                                            wp_sb[:, WPC * pc:WPC * (pc + 1), :],
                                            wproj8[:, WPC * pc:WPC * (pc + 1), :])
                                    ni = 4 * (j + 1) if QS == 512 else T // 128
                                    ps_av = psAV.tile([128, QS], F32, tag="av",
                                                      name="ps_av")
                                    ps_den = psDen.tile([128, QS], F32, tag="den",
                                                        name="ps_den")
                                    e_acc = accp.tile([128, QS], BF16, tag="eacc",
                                                      name="e_acc")
                                    qtok = bb * T + j * QS
                                    for u in range(ni // 2):
                                        # paired score tiles share one Exp call
                                        # over [128, 1024] (amortizes Act setup)
                                        ps_s2 = psS.tile([128, 2, QS], F32,
                                                         tag="s2", name="ps_s2")
                                        for hf in range(2):
                                            i = 2 * u + hf
                                            ktok = bb * T + i * 128
                                            nc.tensor.matmul(
                                                ps_s2[:, hf, :],
                                                k_sb[:, h, ktok:ktok + 128],
                                                q_sb[:, h, qtok:qtok + QS],
                                                start=True, stop=True)
                                        e2 = ep.tile([128, 2, QS], BF16, tag="e2",
                                                     name="e2")
                                        nc.scalar.activation(e2[:], ps_s2[:],
                                                             AF.Exp, bias=0.0,
                                                             scale=SCALE)
                                        d0 = 2 * u - (ni - 4)
                                        if d0 >= 0:
                                            nc.vector.tensor_mul(
                                                e2[:], e2[:],
                                                masks_sb[:, d0:d0 + 2, :])
                                        for hf in range(2):
                                            i = 2 * u + hf
                                            # hybrid denominator: diagonal tiles
                                            # accumulate on PE, the rest on DVE
                                            if i < ni - 4:
                                                if i == 0:
                                                    nc.vector.tensor_copy(
                                                        e_acc[:], e2[:, hf, :])
                                                else:
                                                    nc.vector.tensor_add(
                                                        e_acc[:], e_acc[:],
                                                        e2[:, hf, :])
                                            else:
                                                nc.tensor.matmul(
                                                    ps_den[:], ones_bf[:],
                                                    e2[:, hf, :],
                                                    start=(i == ni - 4),
                                                    stop=(i == ni - 1 and ni == 4))
                                            tt = (bb * T + i * 128) // 128
                                            nc.tensor.matmul(
                                                ps_av[:],
                                                v_sb[:, tt, 128 * h:128 * (h + 1)],
                                                e2[:, hf, :],
                                                start=(i == 0), stop=(i == ni - 1))
                                    if ni > 4:
                                        nc.tensor.matmul(ps_den[:], ones_bf[:],
                                                         e_acc[:], start=False,
                                                         stop=True)
                                    rec_bc = asml.tile([128, QS], F32, tag="rec",
                                                       name="rec_bc")
                                    nc.vector.reciprocal(rec_bc[:], ps_den[:])
                                    vtile = valsp.tile([128, QS], F8, tag="vt",
                                                       name="vtile")
                                    nc.vector.tensor_mul(vtile[:], ps_av[:],
                                                         rec_bc[:])
                                    ncol0 = bb * T + j * QS
                                    for part in range(max(1, QS // CH)):
                                        jg = (ncol0 + part * CH) // CH
                                        w = min(CH, QS)
                                        nc.sync.dma_start(
                                            a2a_in[h][128 * jg:128 * (jg + 1), :],
                                            vtile[:, part * w:(part + 1) * w])
                            if h == 0:
                                # h=0 values complete at half-time: overlap the
                                # first AllToAll with the h=1 attention pass
                                if collectives:
                                    nc.gpsimd.collective_compute(
                                        "AllToAll", OP.bypass, replica_groups=RG,
                                        ins=[a2a_in[0].opt()],
                                        outs=[a2a_out[0].opt()])
                                else:
                                    nc.sync.dma_start(a2a_out[0][:], a2a_in[0][:])

                wqkvp.release()
                if collectives:
                    nc.gpsimd.collective_compute(
                        "AllToAll", OP.bypass, replica_groups=RG,
                        ins=[a2a_in[1].opt()], outs=[a2a_out[1].opt()])
                else:
                    # per-block copies: each vf1 sub-load (and the proj b-chain
                    # behind it) starts as soon as its block lands
                    for g in range(N_CORES):
                        nc.sync.dma_start(a2a_out[1][128 * g:128 * (g + 1), :],
                                          a2a_in[1][128 * g:128 * (g + 1), :])

                # ====== Phase C: proj + residual + LN2 stats (own chunk) ======
                w1p = tc.alloc_tile_pool(name="w1", bufs=6)
                w1_tiles = []
                with tc.tile_pool(name="vf", bufs=1) as vfp, \
                     tc.tile_pool(name="pdrain", bufs=3) as pdp, \
                     tc.tile_pool(name="lnC_small", bufs=1) as lnCs, \
                     tc.tile_pool(name="psP", bufs=3, space="PSUM") as psP, \
                     tc.tile_pool(name="psP2", bufs=1, space="PSUM") as psP2:
                    vf_sb = vfp.tile([128, NB, 2, CH], F8, tag="vf",
                                     name="vf_sb")
                    nc.sync.dma_start(
                        vf_sb[:, :, 0, :],
                        a2a_out[0][:].rearrange("(r p) t -> p r t", p=128))
                    for g in range(N_CORES):
                        nc.sync.dma_start(
                            vf_sb[:, g, 1, :],
                            a2a_out[1][128 * g:128 * (g + 1), :])
                    # prefetch the first w1 ring fills from a pool allocated
                    # BEFORE r1_bf/wp exist: a later pool allocation would reuse
                    # their SBUF and inherit a WAR dependency on the LN2 applies
                    for ft in range(6):
                        w1t = w1p.tile([128, NB * 2, 128], F8, tag="w1",
                                       name="w1t")
                        nc.sync.dma_start(w1t[:], wf18[ft])
                        w1_tiles.append(w1t)
                    ps_sum2 = psP2.tile([128, CH], F32, tag="sum2", name="ps_sum2")
                    ps_ssq2 = psP2.tile([128, CH], F32, tag="ssq2", name="ps_ssq2")
                    r1bfp = tc.alloc_tile_pool(name="r1bf", bufs=1)
                    r1_bf = r1bfp.tile([128, NCT, CH], BF16, tag="r1_bf",
                                       name="r1_bf")

                    def ln2_stats(ot):
                        # lag-2 interleave behind the proj loop: r1[ot] is ready
                        # two iterations later, so the PE stats matmuls never
                        # stall on the drain chain
                        if ot % 2 == 0:
                            nc.scalar.activation(r1_bf[:, ot, :], r1_sb[:, ot, :],
                                                 AF.Identity)
                        else:
                            nc.vector.tensor_copy(r1_bf[:, ot, :],
                                                  r1_sb[:, ot, :])
                        sq2 = pdp.tile([128, CH], BF16, tag="sq2", name="sq2")
                        nc.vector.tensor_mul(sq2[:], r1_bf[:, ot, :],
                                             r1_bf[:, ot, :])
                        nc.tensor.matmul(ps_sum2[:], ones_bf[:], r1_bf[:, ot, :],
                                         start=(ot == 0), stop=(ot == NCT - 1))
                        nc.tensor.matmul(ps_ssq2[:], ones_bf[:], sq2[:],
                                         start=(ot == 0), stop=(ot == NCT - 1))

                    for ot in range(NCT):
                        ps_p = psP.tile([128, CH], F32, tag="p", name="ps_p")
                        for b in range(NB):
                            nc.tensor.matmul(
                                ps_p[:],
                                wp_sb[:, (ot * NB + b) * 2:(ot * NB + b) * 2 + 2, :],
                                vf_sb[:, b, :, :],
                                start=(b == 0), stop=(b == NB - 1),
                                perf_mode=PM.DoubleRow)
                        p_t = pdp.tile([128, CH], F32, tag="p_t", name="p_t")
                        nc.scalar.activation(p_t[:], ps_p[:], AF.Identity,
                                             bias=bproj_sb[:, ot, :], scale=RWS)
                        nc.vector.tensor_add(r1_sb[:, ot, :], p_t[:],
                                             x_tile(ot))
                        if ot >= 2:
                            ln2_stats(ot - 2)
                    ln2_stats(NCT - 2)
                    ln2_stats(NCT - 1)
                    rstd2_bf, shift2_bf = _ln_finish(nc, lnCs, ps_sum2, ps_ssq2,
                                                     C, CH, "ln2", lp=nc)
                    tmp2V = r1bfp.tile([128, 11, CH], BF16, tag="ap2V",
                                       name="tmp2V")
                    nc.vector.tensor_mul(
                        tmp2V[:], r1_bf[:, 0:11, :],
                        rstd2_bf[:].unsqueeze(1).broadcast_to([128, 11, CH]))
                    nc.vector.tensor_add(
                        n2_sb[:, 0:11, :], tmp2V[:],
                        shift2_bf[:].unsqueeze(1).broadcast_to([128, 11, CH]))
                    tmp2P = r1bfp.tile([128, 5, CH], BF16, tag="ap2P",
                                       name="tmp2P")
                    nc.gpsimd.tensor_mul(
                        tmp2P[:], r1_bf[:, 11:16, :],
                        rstd2_bf[:].unsqueeze(1).broadcast_to([128, 5, CH]))
                    nc.gpsimd.tensor_add(
                        n2_sb[:, 11:16, :], tmp2P[:],
                        shift2_bf[:].unsqueeze(1).broadcast_to([128, 5, CH]))
                    r1bfp.release()

                # =============== Phase D: FFN1 (fp8 DR) ===============
                with tc.tile_pool(name="hpool", bufs=1) as hpool, \
                     tc.tile_pool(name="psH", bufs=3, space="PSUM") as psH:
                    h_sb = hpool.tile([128, NFT, CH], F8, tag="h_sb",
                                      name="h_sb")
                    for ft in range(NFT):
                        if ft < 6:
                            w1t = w1_tiles[ft]
                        else:
                            w1t = w1p.tile([128, NB * 2, 128], F8, tag="w1",
                                           name="w1t")
                            nc.sync.dma_start(w1t[:], wf18[ft])
                        ps_h = psH.tile([128, CH], F32, tag="h", name="ps_h")
                        for b in range(NB):
                            nc.tensor.matmul(
                                ps_h[:], w1t[:, 2 * b:2 * b + 2, :],
                                n2_sb[:, 2 * b:2 * b + 2, :],
                                start=(b == 0), stop=(b == NB - 1),
                                perf_mode=PM.DoubleRow)
                        nc.scalar.activation(h_sb[:, ft, :], ps_h[:], GELU,
                                             bias=bf1_sb[:, ft, :], scale=RWS)

                    # ============= Phase D2: FFN2 (fp8 DR) =============
                    with tc.tile_pool(name="w2", bufs=3) as w2p, \
                         tc.tile_pool(name="outp", bufs=3) as outp, \
                         tc.tile_pool(name="psF", bufs=2, space="PSUM") as psF:
                        for ot in range(NCT):
                            w2t = w2p.tile([128, NFB * 2, 128], F8, tag="w2",
                                           name="w2t")
                            nc.sync.dma_start(w2t[:], wf28[ot])
                            ps_f = psF.tile([128, CH], F32, tag="f",
                                            name="ps_f")
                            for g in range(NFB):
                                nc.tensor.matmul(
                                    ps_f[:], w2t[:, 2 * g:2 * g + 2, :],
                                    h_sb[:, 2 * g:2 * g + 2, :],
                                    start=(g == 0), stop=(g == NFB - 1),
                                    perf_mode=PM.DoubleRow)
                            f_t = outp.tile([128, CH], F32, tag="f_t",
                                            name="f_t")
                            nc.scalar.activation(f_t[:], ps_f[:], AF.Identity,
                                                 bias=bf2_sb[:, ot, :],
                                                 scale=RWS)
                            o_t = outp.tile([128, CH], F32, tag="o_t",
                                            name="o_t")
                            nc.vector.tensor_add(o_t[:], f_t[:],
                                                 r1_sb[:, ot, :])
                            nc.sync.dma_start(out[128 * ot:128 * (ot + 1), :],
                                              o_t[:])
                w1p.release()
                projw.release()
                n2pool.release()

    nc.compile()
    return nc


# ----------------------------------------------------------------------------
# Host side
# ----------------------------------------------------------------------------

_NC_CACHE = {}


def _get_nc(T=2048):
    if T not in _NC_CACHE:
        _NC_CACHE[T] = build_decoder(T)
    return _NC_CACHE[T]


def _q8(a):
    """Quantize f32 -> fp8 e4m3 bytes with the x32 pre-scale."""
    return (np.ascontiguousarray(a, np.float32) * WS).astype(
        ml_dtypes.float8_e4m3).view(np.uint8)


def _dr_lhsT_flat(W):
    """[K, M] f32 -> DoubleRow lhsT SBUF layout [128, (M/128 * K/256 * 2), 128]."""
    K, M = W.shape
    a = W.reshape(K // 256, 2, 128, M // 128, 128).transpose(2, 3, 0, 1, 4)
    return np.ascontiguousarray(a.reshape(128, (M // 128) * (K // 256) * 2, 128))


def _dr_lhsT_tiles(W):
    """[K, M] f32 -> per-out-tile DoubleRow layout [M/128, 128, K/256 * 2, 128]."""
    K, M = W.shape
    a = W.reshape(K // 256, 2, 128, M // 128, 128).transpose(3, 2, 0, 1, 4)
    return np.ascontiguousarray(a.reshape(M // 128, 128, (K // 256) * 2, 128))


def _bf16(a):
    return np.ascontiguousarray(a).astype(ml_dtypes.bfloat16).view(np.uint16)


def _prep_inputs(x, Wqkv, bqkv, Wproj, bproj, Wf1, bf1, Wf2, bf2,
                 g1, b1, g2, b2):
    """Fold LN affines, slice heads per core, build per-core in_maps."""
    f32 = np.float32
    x = np.asarray(x, f32)
    Bx, T, Cx = x.shape
    NTOK = Bx * T
    CH = NTOK // N_CORES
    Wqkv = np.asarray(Wqkv, f32)
    bqkv = np.asarray(bqkv, f32)
    g1 = np.asarray(g1, f32); b1 = np.asarray(b1, f32)
    g2 = np.asarray(g2, f32); b2 = np.asarray(b2, f32)
    Wqkv_eff = g1[:, None] * Wqkv
    bqkv_eff = b1 @ Wqkv + bqkv
    Wf1 = np.asarray(Wf1, f32)
    bf1v = np.asarray(bf1, f32)
    Wf1_eff = g2[:, None] * Wf1
    bf1_eff = b2 @ Wf1 + bf1v
    Wproj = np.asarray(Wproj, f32)
    bprojv = np.asarray(bproj, f32)
    Wf2 = np.asarray(Wf2, f32)
    bf2v = np.asarray(bf2, f32)

    xt = np.ascontiguousarray(x.reshape(NTOK, Cx).T)        # [C, NT]

    QS = min(512, T)
    masks = np.zeros((128, 4, QS), f32)
    p = np.arange(128)[:, None]
    fcol = np.arange(QS)[None, :]
    for m in range(4):
        masks[:, m, :] = (p <= fcol - 128 * m).astype(f32)

    shared = {
        "wproj8": _q8(_dr_lhsT_flat(Wproj)),
        "bproj": np.ascontiguousarray(
            bprojv.reshape(Cx // 128, 128).T.reshape(128, Cx // 128, 1)),
        "wf18": _q8(_dr_lhsT_tiles(Wf1_eff)),
        "bf1": np.ascontiguousarray(
            bf1_eff.reshape(F // 128, 128).T.reshape(128, F // 128, 1)),
        "wf28": _q8(_dr_lhsT_tiles(Wf2)),
        "bf2": np.ascontiguousarray(
            bf2v.reshape(Cx // 128, 128).T.reshape(128, Cx // 128, 1)),
        "masks": _bf16(masks),
    }
    in_maps = []
    for c in range(N_CORES):
        h0, h1 = 2 * c, 2 * c + 1
        qcols = np.concatenate([h0 * 384 + np.arange(128),
                                h1 * 384 + np.arange(128)])
        kcols = qcols + 128
        vcols = qcols + 256
        m = dict(shared)
        m["xt"] = np.ascontiguousarray(xt[:, c * CH:(c + 1) * CH])
        m["wq8"] = _q8(_dr_lhsT_flat(Wqkv_eff[:, qcols]))
        m["wk8"] = _q8(_dr_lhsT_flat(Wqkv_eff[:, kcols]))
        # v rhs SBUF layout: [128, K/256 * 2, 256]
        wv = Wqkv_eff[:, vcols].reshape(8, 2, 128, 256).transpose(2, 0, 1, 3)
        m["wv8"] = _q8(np.ascontiguousarray(wv.reshape(128, 16, 256)))
        m["bq"] = np.ascontiguousarray(
            bqkv_eff[qcols].reshape(2, 128).T.reshape(128, 2, 1))
        m["bk"] = np.ascontiguousarray(
            bqkv_eff[kcols].reshape(2, 128).T.reshape(128, 2, 1))
        m["bv_bc"] = np.ascontiguousarray(
            np.broadcast_to(bqkv_eff[vcols][None, :], (128, 256)))
        in_maps.append(m)
    return in_maps, (Bx, T, Cx, CH)


def kernel(x, Wqkv, bqkv, Wproj, bproj, Wf1, bf1, Wf2, bf2,
           g1, b1, g2, b2, _trace=False):
    in_maps, (Bx, T, Cx, CH) = _prep_inputs(
        x, Wqkv, bqkv, Wproj, bproj, Wf1, bf1, Wf2, bf2, g1, b1, g2, b2)
    nc = _get_nc(T)
    res = bass_utils.run_bass_kernel_spmd(
        nc, in_maps, core_ids=list(range(N_CORES)), trace=_trace)
    kernel.last_results = res
    NTOK = Bx * T
    out_t = np.empty((NTOK, Cx), np.float32)
    for c in range(N_CORES):
        out_t[c * CH:(c + 1) * CH, :] = res.results[c]["out"].T
    return out_t.reshape(Bx, T, Cx)


# revision 24
# speedup vs baseline: 1.0524x; 1.0107x over previous
"""Trainium2 Bass kernel for a dense decoder block (LN->MHA->res, LN->FFN->res).

Sharding (8 cores, one NEFF, SPMD-uniform addressing):
  - LN1 token-parallel (512-token chunk/core) -> AllGather of normalized acts
    quantized to fp8e4m3 (x32-scaled weights keep fp8 out of subnormals).
  - QKV + attention head-parallel (2 heads/core, causal, unstable softmax --
    exact because masked logits multiply to 0 post-exp).
  - AllToAll redistributes attention values (fp8): head-shards -> token-shards.
  - proj + residual + LN2 + FFN token-parallel with fp8 weights streamed.
  - LN affine params are folded into the following matmul weights on host.

All heavy GEMMs run in fp8e4m3 with MatmulPerfMode.DoubleRow (256-row
contraction per instruction, 0.5 cycles/output column -> 4x f32r MACs/cycle).
Weights are pre-scaled by 32 on host; PSUM drains apply 1/32 via the
activation-engine scale. Attention (scores/softmax/AV) runs in bf16.
LN statistics use the ones-matmul trick in f32r (1 cycle/row).
Activations stay channel-major [C, tokens]; v is produced token-major
directly by swapping matmul operands, so no transposes are needed.
"""

import math

import numpy as np
import ml_dtypes

import concourse.bass as bass
import concourse.mybir as mybir
import concourse.tile as tile
from concourse import bacc
from concourse import bass_utils

F32 = mybir.dt.float32
F32R = mybir.dt.float32r
BF16 = mybir.dt.bfloat16
F8 = mybir.dt.float8e4
AF = mybir.ActivationFunctionType
OP = mybir.AluOpType
PM = mybir.MatmulPerfMode

N_CORES = 8
B = 2
C = 2048
H = 16
HD = 128
F = 8192
NT = B * 2048                       # total tokens (B*T with T=2048)
H_PER_CORE = H // N_CORES           # 2
NCT = C // 128                      # 16 channel tiles
NB = C // 256                       # 8 DoubleRow contraction blocks
NFT = F // 128                      # 64 ffn tiles
NFB = F // 256                      # 32 ffn DoubleRow blocks
EPS = 1e-5
SCALE = 1.0 / math.sqrt(HD)
WS = 32.0                           # host-side weight scale (drains apply 1/WS)
RWS = 1.0 / WS
GELU = AF.Gelu_apprx_tanh


def r32(ap):
    return ap.bitcast(F32R)


def _ln_finish(nc, pool_small, ps_sum, ps_ssq, n_tok, ncols, tagpfx, lp=None):
    """From broadcast sum/sumsq psums produce SBUF rstd/shift [128, ncols].

    With lp set, outputs are bf16 (feeding the bf16 apply path)."""
    odt = BF16 if lp is not None else F32
    mean = pool_small.tile([128, ncols], F32, tag=f"{tagpfx}_mean", name="mean")
    nc.vector.tensor_scalar_mul(mean[:], ps_sum[:], 1.0 / n_tok)
    # msq = (sum/n)^2 on Act, in parallel with the DVE mean
    msq = pool_small.tile([128, ncols], F32, tag=f"{tagpfx}_msq", name="msq")
    nc.scalar.activation(msq[:], ps_sum[:], AF.Square, scale=1.0 / n_tok)
    varp0 = pool_small.tile([128, ncols], F32, tag=f"{tagpfx}_varp0",
                            name="varp0")
    nc.vector.tensor_scalar(varp0[:], ps_ssq[:], 1.0 / n_tok, EPS,
                            op0=OP.mult, op1=OP.add)
    varp = pool_small.tile([128, ncols], F32, tag=f"{tagpfx}_varp", name="varp")
    nc.vector.scalar_tensor_tensor(varp[:], msq[:], -1.0, varp0[:],
                                   op0=OP.mult, op1=OP.add)
    std = pool_small.tile([128, ncols], F32, tag=f"{tagpfx}_std", name="std")
    nc.scalar.sqrt(std[:], varp[:])
    rstd_bc = pool_small.tile([128, ncols], odt, tag=f"{tagpfx}_rstd", name="rstd")
    if lp is not None:
        with lp.allow_low_precision(reason="rstd broadcast feeds fp8 path"):
            nc.vector.reciprocal(rstd_bc[:], std[:])
    else:
        nc.vector.reciprocal(rstd_bc[:], std[:])
    shift_bc = pool_small.tile([128, ncols], odt, tag=f"{tagpfx}_shift", name="shift")
    nc.vector.scalar_tensor_tensor(shift_bc[:], mean[:], -1.0, rstd_bc[:],
                                   op0=OP.mult, op1=OP.mult)
    return rstd_bc, shift_bc


def build_decoder(T=2048, collectives=True):
    """Build the SPMD decoder-block program for seq length T (2048 = real)."""
    NTOK = B * T
    CH = NTOK // N_CORES            # tokens per core chunk (512)
    NQS = max(1, T // 512)          # q slices of 512 per batch elem
    QS = min(512, T)
    NVT = NTOK // 128               # token-major v tiles (32)
    S_SUB = CH // 128               # 128-token subtiles per chunk (4)

    nc = bacc.Bacc("TRN2", target_bir_lowering=False, debug=False,
                   num_devices=N_CORES)

    # ---- I/O ----
    xt = nc.dram_tensor("xt", [C, CH], F32, kind="ExternalInput").ap()
    xt8 = nc.dram_tensor("xt8", [128, NCT, CH], F8, kind="ExternalInput").ap()
    sq8 = nc.dram_tensor("sq8", [128, NCT, CH], F8, kind="ExternalInput").ap()
    xtbf = nc.dram_tensor("xtbf", [128, NCT, CH], BF16, kind="ExternalInput").ap()
    wq8 = nc.dram_tensor("wq8", [128, 2 * NB * 2, 128], F8, kind="ExternalInput").ap()
    wk8 = nc.dram_tensor("wk8", [128, 2 * NB * 2, 128], F8, kind="ExternalInput").ap()
    wv8 = nc.dram_tensor("wv8", [128, NB * 2, 256], F8, kind="ExternalInput").ap()
    bq = nc.dram_tensor("bq", [128, 2, 1], F32, kind="ExternalInput").ap()
    bk = nc.dram_tensor("bk", [128, 2, 1], F32, kind="ExternalInput").ap()
    bv_bc = nc.dram_tensor("bv_bc", [128, 256], F32, kind="ExternalInput").ap()
    wproj8 = nc.dram_tensor("wproj8", [128, NCT * NB * 2, 128], F8,
                            kind="ExternalInput").ap()
    bproj = nc.dram_tensor("bproj", [128, NCT, 1], F32, kind="ExternalInput").ap()
    wf18 = nc.dram_tensor("wf18", [NFT, 128, NB * 2, 128], F8,
                          kind="ExternalInput").ap()
    bf1 = nc.dram_tensor("bf1", [128, NFT, 1], F32, kind="ExternalInput").ap()
    wf28 = nc.dram_tensor("wf28", [NCT, 128, NFB * 2, 128], F8,
                          kind="ExternalInput").ap()
    bf2 = nc.dram_tensor("bf2", [128, NCT, 1], F32, kind="ExternalInput").ap()
    masks = nc.dram_tensor("masks", [128, 4, QS], BF16, kind="ExternalInput").ap()
    out = nc.dram_tensor("out", [C, CH], F32, kind="ExternalOutput").ap()

    RG = [list(range(N_CORES))]

    with tile.TileContext(nc) as tc:
        with tc.tile_pool(name="dram", bufs=1, space="DRAM") as dram, \
             tc.tile_pool(name="persist", bufs=1) as persist:
            n1_bounce = [dram.tile([C // 2, CH], F8, tag=f"n1_bounce{hh}",
                                   name="n1_bounce") for hh in range(2)]
            n1_full = [dram.tile([N_CORES * C // 2, CH], F8, tag=f"n1_full{hh}",
                                 name="n1_full", addr_space="Shared")
                       for hh in range(2)]
            a2a_in = [dram.tile([C // 2, CH], F8, tag=f"a2a_in{h}",
                                name="a2a_in") for h in range(2)]
            a2a_out = [dram.tile([C // 2, CH], F8, tag=f"a2a_out{h}",
                                 name="a2a_out") for h in range(2)]

            # x tiles are the critical path at t=0: issue their DMAs first.
            xt_view = xt.rearrange("(k p) t -> p k t", p=128)
            ones_sq = persist.tile([128, 128], F32, tag="ones_sq", name="ones_sq")
            ones_bf = persist.tile([128, 128], BF16, tag="ones_bf", name="ones_bf")
            ones8 = persist.tile([128, 2, 128], F8, tag="ones8", name="ones8")
            nc.vector.memset(ones_sq[:], 1.0)
            nc.vector.tensor_copy(ones_bf[:], ones_sq[:])
            nc.vector.memset(ones8[:], 1.0)
            masks_sb = persist.tile([128, 4, QS], BF16, tag="masks", name="masks_sb")
            bq_sb = persist.tile([128, 2, 1], F32, tag="bq", name="bq_sb")
            bk_sb = persist.tile([128, 2, 1], F32, tag="bk", name="bk_sb")
            bv_sb = persist.tile([128, 256], F32, tag="bv", name="bv_sb")
            bproj_sb = persist.tile([128, NCT, 1], F32, tag="bproj", name="bproj_sb")
            bf1_sb = persist.tile([128, NFT, 1], F32, tag="bf1", name="bf1_sb")
            bf2_sb = persist.tile([128, NCT, 1], F32, tag="bf2", name="bf2_sb")

            # r1 survives proj -> final residual add; x survives LN1 -> proj.
            r1_sb = persist.tile([128, NCT, CH], F32, tag="r1", name="r1_sb")

            with tc.tile_pool(name="xpool", bufs=1) as xpool:
                # full-precision x only feeds the proj residual; loaded during
                # attention when the DMA pipe is idle
                x_sb = xpool.tile([128, NCT, CH], F32, tag="x_sb", name="x_sb")

                def x_tile(k):
                    return x_sb[:, k, :]
                n2pool = tc.alloc_tile_pool(name="n2pool", bufs=1)
                n2_sb = n2pool.tile([128, NCT, CH], F8, tag="n2_sb",
                                    name="n2_sb")
                projw = tc.alloc_tile_pool(name="projw", bufs=1)
                wp_sb = projw.tile([128, NCT * NB * 2, 128], F8, tag="wp",
                                   name="wp_sb")
                wqkvp = tc.alloc_tile_pool(name="wqkv", bufs=1)
                wq_sb = wqkvp.tile([128, 2 * NB * 2, 128], F8, tag="wq",
                                   name="wq_sb")
                wk_sb = wqkvp.tile([128, 2 * NB * 2, 128], F8, tag="wk",
                                   name="wk_sb")
                wv_sb = wqkvp.tile([128, NB * 2, 256], F8, tag="wv",
                                   name="wv_sb")

                # ================= Phase A: LN1 on own chunk =================
                xstat = tc.alloc_tile_pool(name="xstat", bufs=1)
                # separate quarter tiles dodge the tile-granular WAR tracking
                x8_q = [xstat.tile([128, 4, CH], F8, tag=f"x8_q{q}", name="x8_q")
                        for q in range(4)]
                s8_q = [xstat.tile([128, 4, CH], F8, tag=f"s8_q{q}", name="s8_q")
                        for q in range(4)]
                xbf_q = [xstat.tile([128, 4, CH], BF16, tag=f"xbf_q{q}",
                         name="xbf_q") for q in range(4)]
                with tc.tile_pool(name="lnA", bufs=3) as lnA, \
                     tc.tile_pool(name="lnA_small", bufs=1) as lnAs, \
                     tc.tile_pool(name="n1pool", bufs=2) as n1pool, \
                     tc.tile_pool(name="psA", bufs=1, space="PSUM") as psA:
                    ps_sum = psA.tile([128, CH], F32, tag="sum", name="ps_sum")
                    ps_ssq = psA.tile([128, CH], F32, tag="ssq", name="ps_ssq")
                    for q in range(4):
                        nc.sync.dma_start(x8_q[q][:], xt8[:, 4 * q:4 * (q + 1), :])
                        nc.sync.dma_start(s8_q[q][:], sq8[:, 4 * q:4 * (q + 1), :])
                    for q in range(4):
                        nc.sync.dma_start(xbf_q[q][:],
                                          xtbf[:, 4 * q:4 * (q + 1), :])
                    nc.sync.dma_start(wq_sb[:], wq8)
                    nc.sync.dma_start(wk_sb[:], wk8)
                    nc.sync.dma_start(wv_sb[:], wv8)
                    nc.sync.dma_start(bq_sb[:], bq)
                    nc.sync.dma_start(bk_sb[:], bk)
                    nc.sync.dma_start(bv_sb[:], bv_bc)
                    nc.sync.dma_start(masks_sb[:], masks)
                    nc.sync.dma_start(bproj_sb[:], bproj)
                    nc.sync.dma_start(bf1_sb[:], bf1)
                    nc.sync.dma_start(bf2_sb[:], bf2)
                    # LN1 stats as fp8 DoubleRow matmuls: host pre-casts x and
                    # x^2, so two k-tiles are summed per instruction
                    for q in range(4):
                        for pp in range(2):
                            st = (q == 0 and pp == 0)
                            sp = (q == 3 and pp == 1)
                            nc.tensor.matmul(
                                ps_sum[:], ones8[:],
                                x8_q[q][:, 2 * pp:2 * pp + 2, :],
                                start=st, stop=sp, perf_mode=PM.DoubleRow)
                            nc.tensor.matmul(
                                ps_ssq[:], ones8[:],
                                s8_q[q][:, 2 * pp:2 * pp + 2, :],
                                start=st, stop=sp, perf_mode=PM.DoubleRow)
                    rstd_bf, shift_bf = _ln_finish(nc, lnAs, ps_sum, ps_ssq,
                                                   C, CH, "ln1", lp=nc)
                    n1_views = [n1_bounce[hh][:].rearrange("(k p) t -> p k t",
                                                           p=128)
                                for hh in range(2)]
                    # batched applies: one big strided op per engine per half
                    # (DVE 6 tiles, Pool 2) instead of 32 per-tile ops
                    for hh in range(2):
                        q0 = 2 * hh
                        n1s = n1pool.tile([128, 8, CH], F8, tag="n1s",
                                          name="n1s")
                        for qq in range(2):
                            q = q0 + qq
                            tmpV = lnA.tile([128, 3, CH], BF16, tag="apV",
                                            name="tmpV")
                            nc.vector.tensor_mul(
                                tmpV[:], xbf_q[q][:, 0:3, :],
                                rstd_bf[:].unsqueeze(1).broadcast_to([128, 3, CH]))
                            nc.vector.tensor_add(
                                n1s[:, 4 * qq:4 * qq + 3, :], tmpV[:],
                                shift_bf[:].unsqueeze(1).broadcast_to([128, 3, CH]))
                            tmpP = lnA.tile([128, 1, CH], BF16, tag="apP",
                                            name="tmpP")
                            nc.gpsimd.tensor_mul(
                                tmpP[:], xbf_q[q][:, 3:4, :],
                                rstd_bf[:].unsqueeze(1).broadcast_to([128, 1, CH]))
                            nc.gpsimd.tensor_add(
                                n1s[:, 4 * qq + 3:4 * qq + 4, :], tmpP[:],
                                shift_bf[:].unsqueeze(1).broadcast_to([128, 1, CH]))
                        nc.sync.dma_start(n1_views[hh][:], n1s[:])
                xstat.release()

                for hh in range(2):
                    if collectives:
                        nc.gpsimd.collective_compute(
                            "AllGather", OP.bypass, replica_groups=RG,
                            ins=[n1_bounce[hh].opt()], outs=[n1_full[hh].opt()])
                    else:  # timing variant: plain copy keeps the dependency edge
                        nc.sync.dma_start(n1_full[hh][0:C // 2, :],
                                          n1_bounce[hh][:])

                # ====== Phase B: QKV (all tokens, own 2 heads, fp8 DR) ======
                with tc.tile_pool(name="qkv_sb", bufs=1) as qkvp:
                    q_sb = qkvp.tile([128, 2, NTOK], BF16, tag="q_sb", name="q_sb")
                    k_sb = qkvp.tile([128, 2, NTOK], BF16, tag="k_sb", name="k_sb")
                    v_sb = qkvp.tile([128, NVT, 256], BF16, tag="v_sb", name="v_sb")

                    with tc.tile_pool(name="n1t", bufs=3) as n1tp, \
                         tc.tile_pool(name="psQK", bufs=1, space="PSUM") as psQK, \
                         tc.tile_pool(name="psV", bufs=1, space="PSUM") as psV:
                        nf_views = [n1_full[hh][:].rearrange(
                            "(r k p) t -> r p k t", r=N_CORES, p=128)
                            for hh in range(2)]
                        for r in range(N_CORES):
                            n1ca = n1tp.tile([128, NCT // 2, CH], F8, tag="n1ca",
                                             name="n1ca")
                            nc.sync.dma_start(n1ca[:], nf_views[0][r])
                            n1cb = n1tp.tile([128, NCT // 2, CH], F8, tag="n1cb",
                                             name="n1cb")
                            nc.sync.dma_start(n1cb[:], nf_views[1][r])
                            ps_q = [psQK.tile([128, CH], F32, tag=f"q{o}",
                                              name=f"ps_q{o}") for o in range(2)]
                            ps_k = [psQK.tile([128, CH], F32, tag=f"k{o}",
                                              name=f"ps_k{o}") for o in range(2)]
                            ps_v = [psV.tile([128, 256], F32, tag=f"v{s}",
                                             name=f"ps_v{s}") for s in range(S_SUB)]
                            for b in range(NB):
                                n1c = n1ca if b < NB // 2 else n1cb
                                bl = b % (NB // 2)
                                rhs = n1c[:, 2 * bl:2 * bl + 2, :]
                                st, sp = (b == 0), (b == NB - 1)
                                for o in range(2):
                                    nc.tensor.matmul(
                                        ps_q[o][:],
                                        wq_sb[:, (o * NB + b) * 2:(o * NB + b) * 2 + 2, :],
                                        rhs, start=st, stop=sp, perf_mode=PM.DoubleRow)
                                    nc.tensor.matmul(
                                        ps_k[o][:],
                                        wk_sb[:, (o * NB + b) * 2:(o * NB + b) * 2 + 2, :],
                                        rhs, start=st, stop=sp, perf_mode=PM.DoubleRow)
                                for s in range(S_SUB):
                                    nc.tensor.matmul(
                                        ps_v[s][:],
                                        n1c[:, 2 * bl:2 * bl + 2, 128 * s:128 * (s + 1)],
                                        wv_sb[:, 2 * b:2 * b + 2, :],
                                        start=st, stop=sp, perf_mode=PM.DoubleRow)
                            for o in range(2):
                                nc.vector.tensor_scalar(
                                    q_sb[:, o, CH * r:CH * (r + 1)], ps_q[o][:],
                                    RWS, bq_sb[:, o, :], op0=OP.mult, op1=OP.add)
                                nc.vector.tensor_scalar(
                                    k_sb[:, o, CH * r:CH * (r + 1)], ps_k[o][:],
                                    RWS, bk_sb[:, o, :], op0=OP.mult, op1=OP.add)
                            for s in range(S_SUB):
                                nc.vector.scalar_tensor_tensor(
                                    v_sb[:, r * S_SUB + s, :], ps_v[s][:], RWS,
                                    bv_sb[:], op0=OP.mult, op1=OP.add)

                    # ========= Phase B2: attention per (head, batch) =========
                    WPC = NCT * NB * 2 // 8
                    wp_pieces = iter(range(8))
                    gated = [False]
                    with tc.tile_pool(name="attn_e", bufs=4) as ep, \
                         tc.tile_pool(name="attn_acc", bufs=2) as accp, \
                         tc.tile_pool(name="attn_small", bufs=3) as asml, \
                         tc.tile_pool(name="vals", bufs=3) as valsp, \
                         tc.tile_pool(name="psS", bufs=3, space="PSUM") as psS, \
                         tc.tile_pool(name="psAV", bufs=1, space="PSUM") as psAV, \
                         tc.tile_pool(name="psDen", bufs=1, space="PSUM") as psDen:
                        for h in range(H_PER_CORE):
                            for bb in range(B):
                                for j in range(NQS):
                                    # stream the proj weights + f32 x during
                                    # attention; the WAW memset gate keeps these
                                    # dep-free loads from grabbing the DMA pipe
                                    # during the LN1/QKV critical window
                                    if not gated[0]:
                                        gated[0] = True
                                        nc.vector.memset(wp_sb[0:1, 0:1, 0:1], 0.0)
                                        nc.vector.memset(x_sb[0:1, 0:1, 0:1], 0.0)
                                    pc = next(wp_pieces, None)
                                    if pc is not None:
                                        nc.sync.dma_start(
                                            wp_sb[:, WPC * pc:WPC * (pc + 1), :],
                                            wproj8[:, WPC * pc:WPC * (pc + 1), :])
                                        nc.sync.dma_start(
                                            x_sb[:, 2 * pc:2 * pc + 2, :],
                                            xt_view[:, 2 * pc:2 * pc + 2, :])
                                    ni = 4 * (j + 1) if QS == 512 else T // 128
                                    ps_av = psAV.tile([128, QS], F32, tag="av",
                                                      name="ps_av")
                                    ps_den = psDen.tile([128, QS], F32, tag="den",
                                                        name="ps_den")
                                    e_acc = accp.tile([128, QS], BF16, tag="eacc",
                                                      name="e_acc")
                                    qtok = bb * T + j * QS
                                    for u in range(ni // 2):
                                        # paired score tiles share one Exp call
                                        # over [128, 1024] (amortizes Act setup)
                                        ps_s2 = psS.tile([128, 2, QS], F32,
                                                         tag="s2", name="ps_s2")
                                        for hf in range(2):
                                            i = 2 * u + hf
                                            ktok = bb * T + i * 128
                                            nc.tensor.matmul(
                                                ps_s2[:, hf, :],
                                                k_sb[:, h, ktok:ktok + 128],
                                                q_sb[:, h, qtok:qtok + QS],
                                                start=True, stop=True)
                                        e2 = ep.tile([128, 2, QS], BF16, tag="e2",
                                                     name="e2")
                                        nc.scalar.activation(e2[:], ps_s2[:],
                                                             AF.Exp, bias=0.0,
                                                             scale=SCALE)
                                        d0 = 2 * u - (ni - 4)
                                        if d0 >= 0:
                                            nc.vector.tensor_mul(
                                                e2[:], e2[:],
                                                masks_sb[:, d0:d0 + 2, :])
                                        for hf in range(2):
                                            i = 2 * u + hf
                                            # hybrid denominator: diagonal tiles
                                            # accumulate on PE, the rest on DVE
                                            if i < ni - 4:
                                                if i == 0:
                                                    nc.vector.tensor_copy(
                                                        e_acc[:], e2[:, hf, :])
                                                else:
                                                    nc.vector.tensor_add(
                                                        e_acc[:], e_acc[:],
                                                        e2[:, hf, :])
                                            else:
                                                nc.tensor.matmul(
                                                    ps_den[:], ones_bf[:],
                                                    e2[:, hf, :],
                                                    start=(i == ni - 4),
                                                    stop=(i == ni - 1 and ni == 4))
                                            tt = (bb * T + i * 128) // 128
                                            nc.tensor.matmul(
                                                ps_av[:],
                                                v_sb[:, tt, 128 * h:128 * (h + 1)],
                                                e2[:, hf, :],
                                                start=(i == 0), stop=(i == ni - 1))
                                    if ni > 4:
                                        nc.tensor.matmul(ps_den[:], ones_bf[:],
                                                         e_acc[:], start=False,
                                                         stop=True)
                                    rec_bc = asml.tile([128, QS], F32, tag="rec",
                                                       name="rec_bc")
                                    nc.vector.reciprocal(rec_bc[:], ps_den[:])
                                    vtile = valsp.tile([128, QS], F8, tag="vt",
                                                       name="vtile")
                                    nc.vector.tensor_mul(vtile[:], ps_av[:],
                                                         rec_bc[:])
                                    ncol0 = bb * T + j * QS
                                    for part in range(max(1, QS // CH)):
                                        jg = (ncol0 + part * CH) // CH
                                        w = min(CH, QS)
                                        nc.sync.dma_start(
                                            a2a_in[h][128 * jg:128 * (jg + 1), :],
                                            vtile[:, part * w:(part + 1) * w])
                            if h == 0:
                                # h=0 values complete at half-time: overlap the
                                # first AllToAll with the h=1 attention pass
                                if collectives:
                                    nc.gpsimd.collective_compute(
                                        "AllToAll", OP.bypass, replica_groups=RG,
                                        ins=[a2a_in[0].opt()],
                                        outs=[a2a_out[0].opt()])
                                else:
                                    nc.sync.dma_start(a2a_out[0][:], a2a_in[0][:])

                wqkvp.release()
                if collectives:
                    nc.gpsimd.collective_compute(
                        "AllToAll", OP.bypass, replica_groups=RG,
                        ins=[a2a_in[1].opt()], outs=[a2a_out[1].opt()])
                else:
                    # per-block copies: each vf1 sub-load (and the proj b-chain
                    # behind it) starts as soon as its block lands
                    for g in range(N_CORES):
                        nc.sync.dma_start(a2a_out[1][128 * g:128 * (g + 1), :],
                                          a2a_in[1][128 * g:128 * (g + 1), :])

                # ====== Phase C: proj + residual + LN2 stats (own chunk) ======
                w1p = tc.alloc_tile_pool(name="w1", bufs=6)
                w1_tiles = []
                with tc.tile_pool(name="vf", bufs=1) as vfp, \
                     tc.tile_pool(name="pdrain", bufs=3) as pdp, \
                     tc.tile_pool(name="lnC_small", bufs=1) as lnCs, \
                     tc.tile_pool(name="psP", bufs=3, space="PSUM") as psP, \
                     tc.tile_pool(name="psP2", bufs=1, space="PSUM") as psP2:
                    vf_sb = vfp.tile([128, NB, 2, CH], F8, tag="vf",
                                     name="vf_sb")
                    nc.sync.dma_start(
                        vf_sb[:, :, 0, :],
                        a2a_out[0][:].rearrange("(r p) t -> p r t", p=128))
                    for g in range(N_CORES):
                        nc.sync.dma_start(
                            vf_sb[:, g, 1, :],
                            a2a_out[1][128 * g:128 * (g + 1), :])
                    # prefetch the first w1 ring fills from a pool allocated
                    # BEFORE r1_bf/wp exist: a later pool allocation would reuse
                    # their SBUF and inherit a WAR dependency on the LN2 applies
                    for ft in range(6):
                        w1t = w1p.tile([128, NB * 2, 128], F8, tag="w1",
                                       name="w1t")
                        nc.sync.dma_start(w1t[:], wf18[ft])
                        w1_tiles.append(w1t)
                    ps_sum2 = psP2.tile([128, CH], F32, tag="sum2", name="ps_sum2")
                    ps_ssq2 = psP2.tile([128, CH], F32, tag="ssq2", name="ps_ssq2")
                    r1bfp = tc.alloc_tile_pool(name="r1bf", bufs=1)
                    r1_bf = r1bfp.tile([128, NCT, CH], BF16, tag="r1_bf",
                                       name="r1_bf")

                    def ln2_stats(ot):
                        # lag-2 interleave behind the proj loop: r1[ot] is ready
                        # two iterations later, so the PE stats matmuls never
                        # stall on the drain chain
                        if ot % 2 == 0:
                            nc.scalar.activation(r1_bf[:, ot, :], r1_sb[:, ot, :],
                                                 AF.Identity)
                        else:
                            nc.vector.tensor_copy(r1_bf[:, ot, :],
                                                  r1_sb[:, ot, :])
                        sq2 = pdp.tile([128, CH], BF16, tag="sq2", name="sq2")
                        nc.vector.tensor_mul(sq2[:], r1_bf[:, ot, :],
                                             r1_bf[:, ot, :])
                        nc.tensor.matmul(ps_sum2[:], ones_bf[:], r1_bf[:, ot, :],
                                         start=(ot == 0), stop=(ot == NCT - 1))
                        nc.tensor.matmul(ps_ssq2[:], ones_bf[:], sq2[:],
                                         start=(ot == 0), stop=(ot == NCT - 1))

                    for ot in range(NCT):
                        ps_p = psP.tile([128, CH], F32, tag="p", name="ps_p")
                        for b in range(NB):
                            nc.tensor.matmul(
                                ps_p[:],
                                wp_sb[:, (ot * NB + b) * 2:(ot * NB + b) * 2 + 2, :],
                                vf_sb[:, b, :, :],
                                start=(b == 0), stop=(b == NB - 1),
                                perf_mode=PM.DoubleRow)
                        p_t = pdp.tile([128, CH], F32, tag="p_t", name="p_t")
                        nc.scalar.activation(p_t[:], ps_p[:], AF.Identity,
                                             bias=bproj_sb[:, ot, :], scale=RWS)
                        nc.vector.tensor_add(r1_sb[:, ot, :], p_t[:],
                                             x_tile(ot))
                        if ot >= 2:
                            ln2_stats(ot - 2)
                    ln2_stats(NCT - 2)
                    ln2_stats(NCT - 1)
                    rstd2_bf, shift2_bf = _ln_finish(nc, lnCs, ps_sum2, ps_ssq2,
                                                     C, CH, "ln2", lp=nc)
                    tmp2V = r1bfp.tile([128, 11, CH], BF16, tag="ap2V",
                                       name="tmp2V")
                    nc.vector.tensor_mul(
                        tmp2V[:], r1_bf[:, 0:11, :],
                        rstd2_bf[:].unsqueeze(1).broadcast_to([128, 11, CH]))
                    nc.vector.tensor_add(
                        n2_sb[:, 0:11, :], tmp2V[:],
                        shift2_bf[:].unsqueeze(1).broadcast_to([128, 11, CH]))
                    tmp2P = r1bfp.tile([128, 5, CH], BF16, tag="ap2P",
                                       name="tmp2P")
                    nc.gpsimd.tensor_mul(
                        tmp2P[:], r1_bf[:, 11:16, :],
                        rstd2_bf[:].unsqueeze(1).broadcast_to([128, 5, CH]))
                    nc.gpsimd.tensor_add(
                        n2_sb[:, 11:16, :], tmp2P[:],
                        shift2_bf[:].unsqueeze(1).broadcast_to([128, 5, CH]))
                    r1bfp.release()

                # =============== Phase D: FFN1 (fp8 DR) ===============
                with tc.tile_pool(name="hpool", bufs=1) as hpool, \
                     tc.tile_pool(name="psH", bufs=3, space="PSUM") as psH:
                    h_sb = hpool.tile([128, NFT, CH], F8, tag="h_sb",
                                      name="h_sb")
                    for ft in range(NFT):
                        if ft < 6:
                            w1t = w1_tiles[ft]
                        else:
                            w1t = w1p.tile([128, NB * 2, 128], F8, tag="w1",
                                           name="w1t")
                            nc.sync.dma_start(w1t[:], wf18[ft])
                        ps_h = psH.tile([128, CH], F32, tag="h", name="ps_h")
                        for b in range(NB):
                            nc.tensor.matmul(
                                ps_h[:], w1t[:, 2 * b:2 * b + 2, :],
                                n2_sb[:, 2 * b:2 * b + 2, :],
                                start=(b == 0), stop=(b == NB - 1),
                                perf_mode=PM.DoubleRow)
                        nc.scalar.activation(h_sb[:, ft, :], ps_h[:], GELU,
                                             bias=bf1_sb[:, ft, :], scale=RWS)

                    # ============= Phase D2: FFN2 (fp8 DR) =============
                    with tc.tile_pool(name="w2", bufs=3) as w2p, \
                         tc.tile_pool(name="outp", bufs=3) as outp, \
                         tc.tile_pool(name="psF", bufs=2, space="PSUM") as psF:
                        for ot in range(NCT):
                            w2t = w2p.tile([128, NFB * 2, 128], F8, tag="w2",
                                           name="w2t")
                            nc.sync.dma_start(w2t[:], wf28[ot])
                            ps_f = psF.tile([128, CH], F32, tag="f",
                                            name="ps_f")
                            for g in range(NFB):
                                nc.tensor.matmul(
                                    ps_f[:], w2t[:, 2 * g:2 * g + 2, :],
                                    h_sb[:, 2 * g:2 * g + 2, :],
                                    start=(g == 0), stop=(g == NFB - 1),
                                    perf_mode=PM.DoubleRow)
                            f_t = outp.tile([128, CH], F32, tag="f_t",
                                            name="f_t")
                            nc.scalar.activation(f_t[:], ps_f[:], AF.Identity,
                                                 bias=bf2_sb[:, ot, :],
                                                 scale=RWS)
                            o_t = outp.tile([128, CH], F32, tag="o_t",
                                            name="o_t")
                            nc.vector.tensor_add(o_t[:], f_t[:],
                                                 r1_sb[:, ot, :])
                            nc.sync.dma_start(out[128 * ot:128 * (ot + 1), :],
                                              o_t[:])
                w1p.release()
                projw.release()
                n2pool.release()

    nc.compile()
    return nc


# ----------------------------------------------------------------------------
# Host side
# ----------------------------------------------------------------------------

_NC_CACHE = {}


def _get_nc(T=2048):
    if T not in _NC_CACHE:
        _NC_CACHE[T] = build_decoder(T)
    return _NC_CACHE[T]


def _q8(a):
    """Quantize f32 -> fp8 e4m3 bytes with the x32 pre-scale."""
    return (np.ascontiguousarray(a, np.float32) * WS).astype(
        ml_dtypes.float8_e4m3).view(np.uint8)


def _dr_lhsT_flat(W):
    """[K, M] f32 -> DoubleRow lhsT SBUF layout [128, (M/128 * K/256 * 2), 128]."""
    K, M = W.shape
    a = W.reshape(K // 256, 2, 128, M // 128, 128).transpose(2, 3, 0, 1, 4)
    return np.ascontiguousarray(a.reshape(128, (M // 128) * (K // 256) * 2, 128))


def _dr_lhsT_tiles(W):
    """[K, M] f32 -> per-out-tile DoubleRow layout [M/128, 128, K/256 * 2, 128]."""
    K, M = W.shape
    a = W.reshape(K // 256, 2, 128, M // 128, 128).transpose(3, 2, 0, 1, 4)
    return np.ascontiguousarray(a.reshape(M // 128, 128, (K // 256) * 2, 128))


def _bf16(a):
    return np.ascontiguousarray(a).astype(ml_dtypes.bfloat16).view(np.uint16)


def _prep_inputs(x, Wqkv, bqkv, Wproj, bproj, Wf1, bf1, Wf2, bf2,
                 g1, b1, g2, b2):
    """Fold LN affines, slice heads per core, build per-core in_maps."""
    f32 = np.float32
    x = np.asarray(x, f32)
    Bx, T, Cx = x.shape
    NTOK = Bx * T
    CH = NTOK // N_CORES
    Wqkv = np.asarray(Wqkv, f32)
    bqkv = np.asarray(bqkv, f32)
    g1 = np.asarray(g1, f32); b1 = np.asarray(b1, f32)
    g2 = np.asarray(g2, f32); b2 = np.asarray(b2, f32)
    Wqkv_eff = g1[:, None] * Wqkv
    bqkv_eff = b1 @ Wqkv + bqkv
    Wf1 = np.asarray(Wf1, f32)
    bf1v = np.asarray(bf1, f32)
    Wf1_eff = g2[:, None] * Wf1
    bf1_eff = b2 @ Wf1 + bf1v
    Wproj = np.asarray(Wproj, f32)
    bprojv = np.asarray(bproj, f32)
    Wf2 = np.asarray(Wf2, f32)
    bf2v = np.asarray(bf2, f32)

    xt = np.ascontiguousarray(x.reshape(NTOK, Cx).T)        # [C, NT]

    QS = min(512, T)
    masks = np.zeros((128, 4, QS), f32)
    p = np.arange(128)[:, None]
    fcol = np.arange(QS)[None, :]
    for m in range(4):
        masks[:, m, :] = (p <= fcol - 128 * m).astype(f32)

    shared = {
        "wproj8": _q8(_dr_lhsT_flat(Wproj)),
        "bproj": np.ascontiguousarray(
            bprojv.reshape(Cx // 128, 128).T.reshape(128, Cx // 128, 1)),
        "wf18": _q8(_dr_lhsT_tiles(Wf1_eff)),
        "bf1": np.ascontiguousarray(
            bf1_eff.reshape(F // 128, 128).T.reshape(128, F // 128, 1)),
        "wf28": _q8(_dr_lhsT_tiles(Wf2)),
        "bf2": np.ascontiguousarray(
            bf2v.reshape(Cx // 128, 128).T.reshape(128, Cx // 128, 1)),
        "masks": _bf16(masks),
    }
    in_maps = []
    for c in range(N_CORES):
        h0, h1 = 2 * c, 2 * c + 1
        qcols = np.concatenate([h0 * 384 + np.arange(128),
                                h1 * 384 + np.arange(128)])
        kcols = qcols + 128
        vcols = qcols + 256
        m = dict(shared)
        xc = np.ascontiguousarray(xt[:, c * CH:(c + 1) * CH])
        m["xt"] = xc
        xc_t = xc.reshape(Cx // 128, 128, CH).transpose(1, 0, 2)
        m["xt8"] = np.ascontiguousarray(xc_t).astype(
            ml_dtypes.float8_e4m3).view(np.uint8)
        m["sq8"] = np.ascontiguousarray(xc_t * xc_t).astype(
            ml_dtypes.float8_e4m3).view(np.uint8)
        m["xtbf"] = np.ascontiguousarray(xc_t).astype(
            ml_dtypes.bfloat16).view(np.uint16)
        m["wq8"] = _q8(_dr_lhsT_flat(Wqkv_eff[:, qcols]))
        m["wk8"] = _q8(_dr_lhsT_flat(Wqkv_eff[:, kcols]))
        # v rhs SBUF layout: [128, K/256 * 2, 256]
        wv = Wqkv_eff[:, vcols].reshape(8, 2, 128, 256).transpose(2, 0, 1, 3)
        m["wv8"] = _q8(np.ascontiguousarray(wv.reshape(128, 16, 256)))
        m["bq"] = np.ascontiguousarray(
            bqkv_eff[qcols].reshape(2, 128).T.reshape(128, 2, 1))
        m["bk"] = np.ascontiguousarray(
            bqkv_eff[kcols].reshape(2, 128).T.reshape(128, 2, 1))
        m["bv_bc"] = np.ascontiguousarray(
            np.broadcast_to(bqkv_eff[vcols][None, :], (128, 256)))
        in_maps.append(m)
    return in_maps, (Bx, T, Cx, CH)


def kernel(x, Wqkv, bqkv, Wproj, bproj, Wf1, bf1, Wf2, bf2,
           g1, b1, g2, b2, _trace=False):
    in_maps, (Bx, T, Cx, CH) = _prep_inputs(
        x, Wqkv, bqkv, Wproj, bproj, Wf1, bf1, Wf2, bf2, g1, b1, g2, b2)
    nc = _get_nc(T)
    res = bass_utils.run_bass_kernel_spmd(
        nc, in_maps, core_ids=list(range(N_CORES)), trace=_trace)
    kernel.last_results = res
    NTOK = Bx * T
    out_t = np.empty((NTOK, Cx), np.float32)
    for c in range(N_CORES):
        out_t[c * CH:(c + 1) * CH, :] = res.results[c]["out"].T
    return out_t.reshape(Bx, T, Cx)
